# revision 8
# baseline (speedup 1.0000x reference)
"""AdaptiveQuantizer Trainium2 kernel (8 NeuronCores, data-parallel over batch).

Math (per pixel (b,h,w), over C=64 channels):
    fmin/fmax = min/max over channels
    rng  = (fmax + 1e-30) - fmin
    lm1  = 2**bits - 1                (exact, via int shift trick)
    u    = lm1 / rng ;  c2 = -u*fmin ;  v = rng / lm1
    w    = u*f + c2                   in [0, lm1]
    r    = round_half_even(w)         via fp32 +M / -M (M = 1.5*2**23)
    out  = v*r + fmin                 (bf16 out; host casts to f32)

Engine balance (cost-model driven; all per 1M-elem superblock). Container
walrus constraints: Pool accepts only tensor_tensor add/mult (0.42 eff,
16.3us/pass) and tensor_scalar (0.6 eff); Pool stt / tt-max / free-dim
reduce are rejected, so min/max stats are DVE-only.
  * DVE: strided 64-ch min/max reduces (8.7 each, 1x), params, a small
    channel slice of the +c2 pass, bf16 tails *v and +fmin (4.3 each,
    2x_1p)                                                 -> ~29us
  * Pool: the *u pass (tt-mult-bcast 16.3) and most of the +c2 pass
    (tt-add-bcast ~13)                                     -> ~29us
  * ACT: +M (7.0) and -M -> bf16 (7.0) rounding passes + bf16 param copies
  * DMA: f32 loads on the SP HWDGE ring, bf16 stores on the ACT HWDGE ring
    (separate FIFOs, no head-of-line blocking), ~19-23us
"""

import os
import sys
from contextlib import nullcontext

for _p in ("/opt/trn_rl_repo", "/root/.axon_site/_ro/trn_rl_repo"):
    if os.path.isdir(_p) and _p not in sys.path:
        sys.path.insert(0, _p)

import numpy as np

import concourse.bass as bass
import concourse.mybir as mybir
from concourse.bass_utils import run_bass_kernel_spmd
from concourse.tile import TileContext
from concourse.vector_clock import ScopedClock

# Problem shapes (hardcoded per spec)
B_FULL, C, H, W = 16, 64, 256, 256
N_CORES = 8
B_LOC = B_FULL // N_CORES  # images per core
PX = H * W                 # pixels per image
P = 128                    # SBUF partitions
WPP = int(os.environ.get("KWPP", "128"))   # pixels per partition per superblock
F_BUFS = int(os.environ.get("KFBUFS", "4"))
FB_BUFS = int(os.environ.get("KFBBUFS", "3"))
CCH = int(os.environ.get("KCCH", "16"))    # channels per DMA chunk
PRE2DVE = int(os.environ.get("KPRE2DVE", "12"))  # +c2 channels on DVE (rest Pool)
BACKLAG = int(os.environ.get("KBACKLAG", "2"))
M_MAGIC = 12582912.0       # 1.5*2**23: fp32 "+M" add == round-to-nearest-even
AL = mybir.AluOpType
F32 = mybir.dt.float32
I32 = mybir.dt.int32
BF16 = mybir.dt.bfloat16
ACTF = mybir.ActivationFunctionType

_drain_patched = False


def _patch_tile_drain():
    """This container's walrus accepts only ONE sync wait per TPB_CTRL
    instruction; Tile's final drain carries one wait per ticked proc.
    Split them across multiple drains."""
    global _drain_patched
    if _drain_patched:
        return
    _drain_patched = True

    def _patched(self, tick_clock, wait_clock):
        nc = self.nc
        drain_inst = nc.sync.drain()
        wait_clock.add_sem_waits(
            drain_inst.ins, ScopedClock({None: tick_clock.global_clock})
        )
        si = drain_inst.ins.sync_info
        waits = list(si.on_wait) if (si is not None and si.on_wait) else []
        if len(waits) > 1:
            si.on_wait = waits[:1]
            for wchunk in waits[1:]:
                extra = nc.sync.drain()
                esi = extra.ins.sync_info
                if esi is None:
                    extra.ins.sync_info = mybir.SyncInfo(
                        on_wait=[wchunk], on_update=[]
                    )
                else:
                    esi.on_wait = [wchunk]
        nc.all_engine_barrier()
        assert self.sems is not None
        popped = nc._tile_sem_poison_stack.pop()
        assert popped is self._sem_poison
        nc.clear_and_free_semaphores(list(self.sems.allocated().values()))
        nc.all_engine_barrier()

    TileContext._drain_and_barrier = _patched


def _split_sync_waits(nc: bass.Bass, max_waits: int = 1) -> None:
    """This container's walrus rejects instructions carrying more than one
    sync wait. Hoist excess waits onto injected same-engine NOPs placed
    immediately before the instruction (engine program order makes this
    semantically identical)."""
    k = 0
    for bb in nc.main_func.blocks:
        insts = list(bb.instructions)
        out_list = []
        changed = False
        for inst in insts:
            si = inst.sync_info
            waits = list(si.on_wait) if (si is not None and si.on_wait) else []
            if len(waits) > max_waits:
                keep = waits[-max_waits:]
                hoist = waits[:-max_waits]
                for i in range(0, len(hoist), max_waits):
                    nop = mybir.InstNoOp(name=f"WSPL-{k}", ins=[], outs=[])
                    k += 1
                    nop.engine = inst.engine
                    nop.sync_info = mybir.SyncInfo(
                        on_wait=hoist[i : i + max_waits], on_update=[]
                    )
                    out_list.append(nop)
                si.on_wait = keep
                changed = True
            out_list.append(inst)
        if changed:
            bb.instructions.clear()
            for inst in out_list:
                bb.instructions.append(inst)


def _bcast(t, nch, wpp):
    return t[:].rearrange("p (o w) -> p o w", o=1).to_broadcast((P, nch, wpp))


def _front_a(nc, fpool, ppool, feat, bits, b, s, variant, wpp, f_bufs,
             fb_bufs):
    """Loads + DVE stats + params + ACT bf16 param copies."""
    SB_PX = P * wpp
    px0 = s * SB_PX
    F = fpool.tile([P, C * wpp], F32, tag="F", bufs=f_bufs)
    Fv = F[:].rearrange("p (c w) -> p c w", c=C)
    # ---- loads (SP HWDGE ring): 1 MiB chunks, contiguous 512 B runs ----
    for cc in range(0, C, CCH):
        src = feat[b, cc : cc + CCH, px0 : px0 + SB_PX]
        src = src.rearrange("c (p w) -> p c w", p=P)
        nc.sync.dma_start(out=Fv[:, cc : cc + CCH, :], in_=src)
    bt = ppool.tile([P, wpp], I32, tag="bt")
    nc.sync.dma_start(
        out=bt[:],
        in_=bits[b, px0 : px0 + SB_PX].rearrange("(p w) -> p w", p=P),
    )

    Fb = fpool.tile([P, C * wpp], BF16, tag="Fb", bufs=fb_bufs)
    st = {"F": F, "Fv": Fv, "Fb": Fb, "b": b, "px0": px0,
          "vb": None, "fminb": None}
    if variant != "full":
        # dma variant: cheap f32->bf16 copy so the store path is exercised
        nc.vector.tensor_copy(Fb[:], F[:])
        return st

    # ---- channel min/max: two half-channel strided DVE reduces each, so
    # the first starts as soon as the first two DMA chunks land ----
    fmax = ppool.tile([P, wpp], F32, tag="fmax")
    fmin = ppool.tile([P, wpp], F32, tag="fmin")
    tmax = ppool.tile([P, wpp], F32, tag="tmax")
    tmin = ppool.tile([P, wpp], F32, tag="tmin")
    h = C // 2
    nc.vector.tensor_reduce(
        fmax[:], Fv[:, :h, :].rearrange("p c w -> p w c"),
        axis=mybir.AxisListType.X, op=AL.max,
    )
    nc.vector.tensor_reduce(
        fmin[:], Fv[:, :h, :].rearrange("p c w -> p w c"),
        axis=mybir.AxisListType.X, op=AL.min,
    )
    nc.vector.tensor_reduce(
        tmax[:], Fv[:, h:, :].rearrange("p c w -> p w c"),
        axis=mybir.AxisListType.X, op=AL.max,
    )
    nc.vector.tensor_reduce(
        tmin[:], Fv[:, h:, :].rearrange("p c w -> p w c"),
        axis=mybir.AxisListType.X, op=AL.min,
    )
    nc.vector.tensor_tensor(fmax[:], fmax[:], tmax[:], AL.max)
    nc.vector.tensor_tensor(fmin[:], fmin[:], tmin[:], AL.min)

    # ---- lm1 = 2**bits - 1 exactly: (bits+127)<<23 bitcast f32, -1 ----
    lvl_i = ppool.tile([P, wpp], I32, tag="lvl_i")
    nc.vector.tensor_scalar_add(lvl_i[:], bt[:], 127)
    nc.vector.tensor_scalar(lvl_i[:], lvl_i[:], 23, None, AL.logical_shift_left)
    lm1 = ppool.tile([P, wpp], F32, tag="lm1")
    nc.vector.tensor_scalar_add(lm1[:], lvl_i[:].bitcast(F32), -1.0)

    # ---- per-pixel params ([P, wpp] tiles, small DVE ops) ----
    rng = ppool.tile([P, wpp], F32, tag="rng")
    nc.vector.scalar_tensor_tensor(
        rng[:], fmax[:], 1e-30, fmin[:], AL.add, AL.subtract
    )
    rinv = ppool.tile([P, wpp], F32, tag="rinv")
    nc.vector.reciprocal(rinv[:], rng[:])
    u = ppool.tile([P, wpp], F32, tag="u")
    nc.vector.scalar_tensor_tensor(u[:], lm1[:], 0.0, rinv[:], AL.add, AL.mult)
    c2 = ppool.tile([P, wpp], F32, tag="c2")
    nc.vector.scalar_tensor_tensor(c2[:], u[:], -1.0, fmin[:], AL.mult, AL.mult)
    ilm1 = ppool.tile([P, wpp], F32, tag="ilm1")
    nc.vector.reciprocal(ilm1[:], lm1[:])
    v = ppool.tile([P, wpp], F32, tag="v")
    nc.vector.scalar_tensor_tensor(v[:], rng[:], 0.0, ilm1[:], AL.add, AL.mult)
    # bf16 copies of v / fmin for the tails (ACT)
    vb = ppool.tile([P, wpp], BF16, tag="vb")
    nc.scalar.activation(vb[:], v[:], ACTF.Copy, bias=0.0, scale=1.0)
    fminb = ppool.tile([P, wpp], BF16, tag="fminb")
    nc.scalar.activation(fminb[:], fmin[:], ACTF.Copy, bias=0.0, scale=1.0)
    st["u"] = u
    st["c2"] = c2
    st["vb"] = vb
    st["fminb"] = fminb
    return st


def _front_b(nc, st, variant, wpp):
    """Pre-round passes (*u on Pool, +c2 split DVE/Pool) + ACT rounding."""
    if variant != "full":
        return
    F, Fv, Fb = st["F"], st["Fv"], st["Fb"]
    u, c2 = st["u"], st["c2"]
    # *u: Pool tt-mult with broadcast (walrus allows Pool tt add/mult only)
    _p1 = nc.gpsimd.tensor_tensor(Fv, Fv, _bcast(u, C, wpp), AL.mult)
    # +c2: small DVE slice + Pool for the rest, disjoint channel ranges
    k = PRE2DVE
    if k > 0:
        nc.vector.tensor_tensor(
            Fv[:, :k, :], Fv[:, :k, :], _bcast(c2, k, wpp), AL.add
        )
    if k < C:
        _p2 = nc.gpsimd.tensor_tensor(
            Fv[:, k:, :], Fv[:, k:, :], _bcast(c2, C - k, wpp), AL.add
        )
    # ---- rounding on ACT: +M (f32, in place), then -M -> bf16 (exact) ----
    nc.scalar.activation(F[:], F[:], ACTF.Copy, bias=M_MAGIC, scale=1.0)
    nc.scalar.activation(Fb[:], F[:], ACTF.Copy, bias=-M_MAGIC, scale=1.0)


def _back(nc, st, out, wpp):
    """Post-round bf16 tails (DVE 2x tt) + bf16 store on the ACT HWDGE ring."""
    SB_PX = P * wpp
    b, px0 = st["b"], st["px0"]
    Fb = st["Fb"]
    Fbv = Fb[:].rearrange("p (c w) -> p c w", c=C)
    if st["vb"] is not None:
        nc.vector.tensor_tensor(Fbv, Fbv, _bcast(st["vb"], C, wpp), AL.mult)
        nc.vector.tensor_tensor(Fbv, Fbv, _bcast(st["fminb"], C, wpp), AL.add)
    for cc in range(0, C, CCH):
        dst = out[b, cc : cc + CCH, px0 : px0 + SB_PX]
        dst = dst.rearrange("c (p w) -> p c w", p=P)
        nc.scalar.dma_start(out=dst, in_=Fbv[:, cc : cc + CCH, :])


def build(
    reps: int = 1,
    variant: str = "full",
    timed_loop: int = 0,
    wpp: int = None,
    f_bufs: int = None,
) -> bass.Bass:
    """Build the per-core Bass program.

    reps: python-unrolled repetitions of the whole (idempotent) workload.
    variant: full | dma (bisection aid: loads + cast + stores only).
    timed_loop: if >0, build a timing-only program: internal DRAM tensors
    (no input upload), tiny dummy output, and a hardware For_i loop running
    the workload `timed_loop` times.
    """
    _patch_tile_drain()
    if wpp is None:
        wpp = WPP
    if f_bufs is None:
        f_bufs = F_BUFS
    fb_bufs = FB_BUFS
    n_sb = PX // (P * wpp)
    nc = bass.Bass()
    if timed_loop:
        feat = nc.dram_tensor("features_i", [B_LOC, C, PX], F32)
        bits = nc.dram_tensor("bits_i", [B_LOC, PX], I32)
        out = nc.dram_tensor("out_i", [B_LOC, C, PX], BF16)
        dummy = nc.declare_dram_parameter("out", [1, 128], F32, isOutput=True)
    else:
        feat = nc.declare_dram_parameter(
            "features", [B_LOC, C, PX], F32, isOutput=False
        )
        bits = nc.declare_dram_parameter(
            "bit_allocation", [B_LOC, PX], I32, isOutput=False
        )
        out = nc.declare_dram_parameter("out", [B_LOC, C, PX], BF16, isOutput=True)

    with TileContext(nc) as tc:
        with (
            tc.tile_pool(name="fpool", bufs=2) as fpool,
            tc.tile_pool(name="ppool", bufs=2) as ppool,
        ):
            loop_cm = tc.For_i(0, timed_loop, 1) if timed_loop else nullcontext()
            with loop_cm:
                for _rep in range(reps):
                    blocks = [
                        (b, s) for b in range(B_LOC) for s in range(n_sb)
                    ]
                    pend = {}
                    for i in range(len(blocks) + BACKLAG):
                        if i < len(blocks):
                            b, s = blocks[i]
                            pend[i] = _front_a(
                                nc, fpool, ppool, feat, bits, b, s,
                                variant, wpp, f_bufs, fb_bufs,
                            )
                        j = i - BACKLAG
                        if j >= 0:
                            _back(nc, pend[j], out, wpp)
                            del pend[j]
                        if i < len(blocks):
                            _front_b(nc, pend[i], variant, wpp)
            if timed_loop:
                dtile = ppool.tile([1, 128], F32, tag="dummy")
                nc.vector.memset(dtile[:], 0.0)
                nc.sync.dma_start(out=dummy[:], in_=dtile[:])
    _split_sync_waits(nc)
    return nc


_nc_cache: dict = {}


def _get_nc(reps: int = 1, variant: str = "full", timed_loop: int = 0) -> bass.Bass:
    key = (reps, variant, timed_loop)
    if key not in _nc_cache:
        _nc_cache[key] = build(reps, variant, timed_loop)
    return _nc_cache[key]


def _in_maps(features: np.ndarray, bit_allocation: np.ndarray):
    f = np.ascontiguousarray(features, dtype=np.float32).reshape(B_FULL, C, PX)
    ba = np.ascontiguousarray(bit_allocation, dtype=np.int32).reshape(B_FULL, PX)
    maps = []
    for i in range(N_CORES):
        b0 = i * B_LOC
        maps.append(
            {
                "features": f[b0 : b0 + B_LOC],
                "bit_allocation": ba[b0 : b0 + B_LOC],
            }
        )
    return maps


def run(
    features: np.ndarray,
    bit_allocation: np.ndarray,
    reps: int = 1,
    variant: str = "full",
):
    nc = _get_nc(reps, variant)
    maps = _in_maps(features, bit_allocation)
    res = run_bass_kernel_spmd(nc, maps, core_ids=list(range(N_CORES)))
    outs = [
        res.results[i]["out"].astype(np.float32).reshape(B_LOC, C, H, W)
        for i in range(N_CORES)
    ]
    return np.concatenate(outs, axis=0)


def run_timed(timed_loop: int, variant: str = "full"):
    """Run the timing-only program (no input upload); returns nothing useful."""
    nc = _get_nc(1, variant, timed_loop)
    maps = [{} for _ in range(N_CORES)]
    run_bass_kernel_spmd(nc, maps, core_ids=list(range(N_CORES)))


def kernel(features: np.ndarray, bit_allocation: np.ndarray) -> np.ndarray:
    return run(features, bit_allocation, reps=1)


# revision 23
# speedup vs baseline: 1.2255x; 1.2255x over previous
"""AdaptiveQuantizer Trainium2 kernel (8 NeuronCores, data-parallel over batch).

Math (per pixel (b,h,w), over C=64 channels):
    fmin/fmax = min/max over channels
    rng  = (fmax + 1e-30) - fmin
    lm1  = 2**bits - 1                (exact, via int shift trick)
    u    = lm1 / rng ;  c2 = -u*fmin ;  v = rng / lm1
    w    = u*f + c2                   in [0, lm1]
    r    = round_half_even(w)         via fp32 +M / -M (M = 1.5*2**23)
    out  = v*r + fmin                 (bf16 out; host casts to f32)

Engine balance (cost-model driven; all per 1M-elem superblock). Container
walrus constraints: Pool accepts only tensor_tensor add/mult (0.42 eff,
16.3us/pass) and tensor_scalar (0.6 eff); Pool stt / tt-max / free-dim
reduce are rejected, so min/max stats are DVE-only.
  * DVE: strided 64-ch min/max reduces (8.7 each, 1x), params, a small
    channel slice of the +c2 pass, bf16 tails *v and +fmin (4.3 each,
    2x_1p)                                                 -> ~29us
  * Pool: the *u pass (tt-mult-bcast 16.3) and most of the +c2 pass
    (tt-add-bcast ~13)                                     -> ~29us
  * ACT: +M (7.0) and -M -> bf16 (7.0) rounding passes + bf16 param copies
  * DMA: f32 loads on the SP HWDGE ring, bf16 stores on the ACT HWDGE ring
    (separate FIFOs, no head-of-line blocking), ~19-23us
"""

import os
import sys
from contextlib import nullcontext

for _p in ("/opt/trn_rl_repo", "/root/.axon_site/_ro/trn_rl_repo"):
    if os.path.isdir(_p) and _p not in sys.path:
        sys.path.insert(0, _p)

import numpy as np

import concourse.bass as bass
import concourse.mybir as mybir
from concourse.bass_utils import run_bass_kernel_spmd
from concourse.tile import TileContext
from concourse.vector_clock import ScopedClock

# Problem shapes (hardcoded per spec)
B_FULL, C, H, W = 16, 64, 256, 256
N_CORES = 8
B_LOC = B_FULL // N_CORES  # images per core
PX = H * W                 # pixels per image
P = 128                    # SBUF partitions
WPP = int(os.environ.get("KWPP", "128"))   # pixels per partition per superblock
F_BUFS = int(os.environ.get("KFBUFS", "3"))
FB_BUFS = int(os.environ.get("KFBBUFS", "3"))
CCH = int(os.environ.get("KCCH", "16"))    # channels per DMA chunk
PRE2DVE = int(os.environ.get("KPRE2DVE", "32"))  # +c2 channels on DVE (rest Pool)
BACKLAG = int(os.environ.get("KBACKLAG", "2"))
M_MAGIC = 12582912.0       # 1.5*2**23: fp32 "+M" add == round-to-nearest-even
AL = mybir.AluOpType
F32 = mybir.dt.float32
I32 = mybir.dt.int32
BF16 = mybir.dt.bfloat16
FP16 = mybir.dt.float16
ACTF = mybir.ActivationFunctionType

_drain_patched = False


def _patch_tile_drain():
    """This container's walrus accepts only ONE sync wait per TPB_CTRL
    instruction; Tile's final drain carries one wait per ticked proc.
    Split them across multiple drains."""
    global _drain_patched
    if _drain_patched:
        return
    _drain_patched = True

    def _patched(self, tick_clock, wait_clock):
        nc = self.nc
        drain_inst = nc.sync.drain()
        wait_clock.add_sem_waits(
            drain_inst.ins, ScopedClock({None: tick_clock.global_clock})
        )
        si = drain_inst.ins.sync_info
        waits = list(si.on_wait) if (si is not None and si.on_wait) else []
        if len(waits) > 1:
            si.on_wait = waits[:1]
            for wchunk in waits[1:]:
                extra = nc.sync.drain()
                esi = extra.ins.sync_info
                if esi is None:
                    extra.ins.sync_info = mybir.SyncInfo(
                        on_wait=[wchunk], on_update=[]
                    )
                else:
                    esi.on_wait = [wchunk]
        nc.all_engine_barrier()
        assert self.sems is not None
        popped = nc._tile_sem_poison_stack.pop()
        assert popped is self._sem_poison
        nc.clear_and_free_semaphores(list(self.sems.allocated().values()))
        nc.all_engine_barrier()

    TileContext._drain_and_barrier = _patched


def _split_sync_waits(nc: bass.Bass, max_waits: int = 1) -> None:
    """This container's walrus rejects instructions carrying more than one
    sync wait. Hoist excess waits onto injected same-engine NOPs placed
    immediately before the instruction (engine program order makes this
    semantically identical)."""
    k = 0
    for bb in nc.main_func.blocks:
        insts = list(bb.instructions)
        out_list = []
        changed = False
        for inst in insts:
            si = inst.sync_info
            waits = list(si.on_wait) if (si is not None and si.on_wait) else []
            if len(waits) > max_waits:
                keep = waits[-max_waits:]
                hoist = waits[:-max_waits]
                for i in range(0, len(hoist), max_waits):
                    nop = mybir.InstNoOp(name=f"WSPL-{k}", ins=[], outs=[])
                    k += 1
                    nop.engine = inst.engine
                    nop.sync_info = mybir.SyncInfo(
                        on_wait=hoist[i : i + max_waits], on_update=[]
                    )
                    out_list.append(nop)
                si.on_wait = keep
                changed = True
            out_list.append(inst)
        if changed:
            bb.instructions.clear()
            for inst in out_list:
                bb.instructions.append(inst)


def _bcast(t, nch, wpp):
    return t[:].rearrange("p (o w) -> p o w", o=1).to_broadcast((P, nch, wpp))


def _vflags(variant):
    """Stage flags per variant (bisection aids)."""
    full = variant == "full"
    return {
        "stats": full or variant in ("red", "tree"),
        "tree": variant == "tree",
        "pre": "split" if full else (
            "dve" if variant == "pre_dve" else
            "pool" if variant == "pre_pool" else None
        ),
        "round": (
            "act" if full or variant in ("act2", "act2_sp") else
            "dve" if variant == "rounddve" else None
        ),
        "tails": full,
        "sp_store": variant in ("dma_sp", "act2_sp"),
    }


def _emit_stats(nc, ppool, Fv, wpp):
    fmax = ppool.tile([P, wpp], F32, tag="fmax")
    fmin = ppool.tile([P, wpp], F32, tag="fmin")
    tmax = ppool.tile([P, wpp], F32, tag="tmax")
    tmin = ppool.tile([P, wpp], F32, tag="tmin")
    h = C // 2
    nc.vector.tensor_reduce(
        fmax[:], Fv[:, :h, :].rearrange("p c w -> p w c"),
        axis=mybir.AxisListType.X, op=AL.max,
    )
    nc.vector.tensor_reduce(
        fmin[:], Fv[:, :h, :].rearrange("p c w -> p w c"),
        axis=mybir.AxisListType.X, op=AL.min,
    )
    nc.vector.tensor_reduce(
        tmax[:], Fv[:, h:, :].rearrange("p c w -> p w c"),
        axis=mybir.AxisListType.X, op=AL.max,
    )
    nc.vector.tensor_reduce(
        tmin[:], Fv[:, h:, :].rearrange("p c w -> p w c"),
        axis=mybir.AxisListType.X, op=AL.min,
    )
    nc.vector.tensor_tensor(fmax[:], fmax[:], tmax[:], AL.max)
    nc.vector.tensor_tensor(fmin[:], fmin[:], tmin[:], AL.min)
    return fmax, fmin


def _emit_stats_tree(nc, ppool, Fv, wpp):
    """Baseline-style contiguous stt tree (64->32->16->8->4, reduce 4)."""
    fmax = ppool.tile([P, wpp], F32, tag="fmax")
    fmin = ppool.tile([P, wpp], F32, tag="fmin")
    sA = ppool.tile([P, (C // 2) * wpp], F32, tag="sA", bufs=1)
    sB = ppool.tile([P, (C // 4) * wpp], F32, tag="sB", bufs=1)
    vA = sA[:].rearrange("p (c w) -> p c w", c=C // 2)
    vB = sB[:].rearrange("p (c w) -> p c w", c=C // 4)
    for out_t, op in ((fmax, AL.max), (fmin, AL.min)):
        cur = Fv
        nch = C
        views = [vA, vB]
        bi = 0
        while nch > 4:
            half = nch // 2
            dst = views[bi % 2][:, :half, :]
            nc.vector.scalar_tensor_tensor(
                dst, cur[:, :half, :], 0.0, cur[:, half:nch, :], AL.add, op
            )
            cur = dst
            nch = half
            bi += 1
        nc.vector.tensor_reduce(
            out_t[:],
            cur[:, 0:4, :].rearrange("p c w -> p w c"),
            axis=mybir.AxisListType.X,
            op=op,
        )
    return fmax, fmin


def _front_a(nc, fpool, ppool, feat, bits, b, s, variant, wpp, f_bufs,
             fb_bufs):
    """Loads + DVE stats + params + ACT bf16 param copies."""
    fl = _vflags(variant)
    SB_PX = P * wpp
    px0 = s * SB_PX
    F = fpool.tile([P, C * wpp], F32, tag="F", bufs=f_bufs)
    Fv = F[:].rearrange("p (c w) -> p c w", c=C)
    # ---- loads (SP HWDGE ring): 1 MiB chunks, contiguous 512 B runs ----
    for cc in range(0, C, CCH):
        src = feat[b, cc : cc + CCH, px0 : px0 + SB_PX]
        src = src.rearrange("c (p w) -> p c w", p=P)
        nc.sync.dma_start(out=Fv[:, cc : cc + CCH, :], in_=src)
    bt = ppool.tile([P, wpp], I32, tag="bt")
    nc.sync.dma_start(
        out=bt[:],
        in_=bits[b, px0 : px0 + SB_PX].rearrange("(p w) -> p w", p=P),
    )

    Fb = fpool.tile([P, C * wpp], BF16, tag="Fb", bufs=fb_bufs)
    st = {"F": F, "Fv": Fv, "Fb": Fb, "b": b, "px0": px0,
          "vb": None, "fminb": None}
    if variant != "full":
        if fl["pre"] == "dve":
            u = ppool.tile([P, wpp], F32, tag="u")
            c2 = ppool.tile([P, wpp], F32, tag="c2")
            nc.vector.memset(u[:], 1.0)
            nc.vector.memset(c2[:], 0.0)
            nc.vector.tensor_tensor(Fv, Fv, _bcast(u, C, wpp), AL.mult)
            nc.vector.tensor_tensor(Fv, Fv, _bcast(c2, C, wpp), AL.add)
        elif fl["pre"] == "pool":
            u = ppool.tile([P, wpp], F32, tag="u")
            c2 = ppool.tile([P, wpp], F32, tag="c2")
            nc.vector.memset(u[:], 1.0)
            nc.vector.memset(c2[:], 0.0)
            _p1 = nc.gpsimd.tensor_tensor(Fv, Fv, _bcast(u, C, wpp), AL.mult)
            _p2 = nc.gpsimd.tensor_tensor(Fv, Fv, _bcast(c2, C, wpp), AL.add)
        if fl["stats"]:
            if fl["tree"]:
                _emit_stats_tree(nc, ppool, Fv, wpp)
            else:
                _emit_stats(nc, ppool, Fv, wpp)
        if fl["round"] == "act":
            nc.scalar.activation(F[:], F[:], ACTF.Copy, bias=M_MAGIC, scale=1.0)
            nc.scalar.activation(Fb[:], F[:], ACTF.Copy, bias=-M_MAGIC, scale=1.0)
        elif fl["round"] == "dve":
            nc.vector.tensor_scalar(
                Fb[:], F[:], M_MAGIC, M_MAGIC, AL.add, AL.subtract
            )
        else:
            nc.vector.tensor_copy(Fb[:], F[:])
        return st

    # ---- channel min/max: two half-channel strided DVE reduces each, so
    # the first starts as soon as the first two DMA chunks land ----
    fmax, fmin = _emit_stats(nc, ppool, Fv, wpp)

    # ---- lm1 = 2**bits - 1 exactly: (bits+127)<<23 bitcast f32, -1 ----
    lvl_i = ppool.tile([P, wpp], I32, tag="lvl_i")
    nc.vector.tensor_scalar_add(lvl_i[:], bt[:], 127)
    nc.vector.tensor_scalar(lvl_i[:], lvl_i[:], 23, None, AL.logical_shift_left)
    lm1 = ppool.tile([P, wpp], F32, tag="lm1")
    nc.vector.tensor_scalar_add(lm1[:], lvl_i[:].bitcast(F32), -1.0)

    # ---- per-pixel params ([P, wpp] tiles, small DVE ops) ----
    rng = ppool.tile([P, wpp], F32, tag="rng")
    nc.vector.scalar_tensor_tensor(
        rng[:], fmax[:], 1e-30, fmin[:], AL.add, AL.subtract
    )
    rinv = ppool.tile([P, wpp], F32, tag="rinv")
    nc.vector.reciprocal(rinv[:], rng[:])
    u = ppool.tile([P, wpp], F32, tag="u")
    nc.vector.scalar_tensor_tensor(u[:], lm1[:], 0.0, rinv[:], AL.add, AL.mult)
    c2 = ppool.tile([P, wpp], F32, tag="c2")
    nc.vector.scalar_tensor_tensor(c2[:], u[:], -1.0, fmin[:], AL.mult, AL.mult)
    ilm1 = ppool.tile([P, wpp], F32, tag="ilm1")
    nc.vector.reciprocal(ilm1[:], lm1[:])
    v = ppool.tile([P, wpp], F32, tag="v")
    nc.vector.scalar_tensor_tensor(v[:], rng[:], 0.0, ilm1[:], AL.add, AL.mult)
    # bf16 copies of v / fmin for the tails (ACT)
    vb = ppool.tile([P, wpp], BF16, tag="vb")
    nc.scalar.activation(vb[:], v[:], ACTF.Copy, bias=0.0, scale=1.0)
    fminb = ppool.tile([P, wpp], BF16, tag="fminb")
    nc.scalar.activation(fminb[:], fmin[:], ACTF.Copy, bias=0.0, scale=1.0)
    st["u"] = u
    st["c2"] = c2
    st["vb"] = vb
    st["fminb"] = fminb
    return st


def _front_b(nc, st, variant, wpp):
    """Pre-round passes (*u on Pool, +c2 split DVE/Pool) + ACT rounding."""
    if variant != "full":
        return
    F, Fv, Fb = st["F"], st["Fv"], st["Fb"]
    u, c2 = st["u"], st["c2"]
    # *u: Pool tt-mult with broadcast (walrus allows Pool tt add/mult only)
    _p1 = nc.gpsimd.tensor_tensor(Fv, Fv, _bcast(u, C, wpp), AL.mult)
    # +c2: small DVE slice + Pool for the rest, disjoint channel ranges
    k = PRE2DVE
    if k > 0:
        nc.vector.tensor_tensor(
            Fv[:, :k, :], Fv[:, :k, :], _bcast(c2, k, wpp), AL.add
        )
    if k < C:
        _p2 = nc.gpsimd.tensor_tensor(
            Fv[:, k:, :], Fv[:, k:, :], _bcast(c2, C - k, wpp), AL.add
        )
    # ---- rounding on ACT: +M (f32, in place), then -M -> bf16 (exact) ----
    nc.scalar.activation(F[:], F[:], ACTF.Copy, bias=M_MAGIC, scale=1.0)
    nc.scalar.activation(Fb[:], F[:], ACTF.Copy, bias=-M_MAGIC, scale=1.0)


def _back(nc, st, out, variant, wpp):
    """Post-round bf16 tails (DVE 2x tt) + bf16 store on the ACT HWDGE ring."""
    SB_PX = P * wpp
    b, px0 = st["b"], st["px0"]
    Fb = st["Fb"]
    Fbv = Fb[:].rearrange("p (c w) -> p c w", c=C)
    if st["vb"] is not None:
        nc.vector.tensor_tensor(Fbv, Fbv, _bcast(st["vb"], C, wpp), AL.mult)
        nc.vector.tensor_tensor(Fbv, Fbv, _bcast(st["fminb"], C, wpp), AL.add)
    eng = nc.sync if _vflags(variant)["sp_store"] else nc.scalar
    for cc in range(0, C, CCH):
        dst = out[b, cc : cc + CCH, px0 : px0 + SB_PX]
        dst = dst.rearrange("c (p w) -> p c w", p=P)
        eng.dma_start(out=dst, in_=Fbv[:, cc : cc + CCH, :])


def _w_loads(nc, fpool, ppool, feat, bits, b, s, wpp, f_bufs):
    """Stage A: SP-ring loads for superblock (b, s)."""
    SB_PX = P * wpp
    px0 = s * SB_PX
    F = fpool.tile([P, C * wpp], F32, tag="F", bufs=f_bufs)
    Fv = F[:].rearrange("p (c w) -> p c w", c=C)
    for cc in range(0, C, CCH):
        src = feat[b, cc : cc + CCH, px0 : px0 + SB_PX]
        src = src.rearrange("c (p w) -> p c w", p=P)
        nc.sync.dma_start(out=Fv[:, cc : cc + CCH, :], in_=src)
    bt = ppool.tile([P, wpp], I32, tag="bt")
    nc.sync.dma_start(
        out=bt[:],
        in_=bits[b, px0 : px0 + SB_PX].rearrange("(p w) -> p w", p=P),
    )
    return {"F": F, "Fv": Fv, "bt": bt, "b": b, "px0": px0}


def _w_stats(nc, fpool, ppool, st, wpp):
    """Stage B: ACT f32->fp16 cast + DVE fp16 min/max tree + params."""
    F, Fv, bt = st["F"], st["Fv"], st["bt"]
    Fh = fpool.tile([P, C * wpp], FP16, tag="Fh", bufs=2)
    nc.scalar.activation(Fh[:], F[:], ACTF.Copy, bias=0.0, scale=1.0)
    Fhv = Fh[:].rearrange("p (c w) -> p c w", c=C)
    fmaxh = ppool.tile([P, wpp], FP16, tag="fmaxh")
    fminh = ppool.tile([P, wpp], FP16, tag="fminh")
    sA = ppool.tile([P, (C // 2) * wpp], FP16, tag="sA", bufs=1)
    sB = ppool.tile([P, (C // 4) * wpp], FP16, tag="sB", bufs=1)
    vA = sA[:].rearrange("p (c w) -> p c w", c=C // 2)
    vB = sB[:].rearrange("p (c w) -> p c w", c=C // 4)
    for out_t, op in ((fmaxh, AL.max), (fminh, AL.min)):
        cur = Fhv
        nch = C
        views = [vA, vB]
        bi = 0
        while nch > 4:
            half = nch // 2
            dst = views[bi % 2][:, :half, :]
            nc.vector.tensor_tensor(dst, cur[:, :half, :], cur[:, half:nch, :], op)
            cur = dst
            nch = half
            bi += 1
        nc.vector.tensor_reduce(
            out_t[:], cur[:, 0:4, :].rearrange("p c w -> p w c"),
            axis=mybir.AxisListType.X, op=op,
        )
    # lm1 = 2**bits - 1 exactly
    lvl_i = ppool.tile([P, wpp], I32, tag="lvl_i")
    nc.vector.tensor_scalar_add(lvl_i[:], bt[:], 127)
    nc.vector.tensor_scalar(lvl_i[:], lvl_i[:], 23, None, AL.logical_shift_left)
    lm1 = ppool.tile([P, wpp], F32, tag="lm1")
    nc.vector.tensor_scalar_add(lm1[:], lvl_i[:].bitcast(F32), -1.0)
    # per-pixel params (f32 [P, wpp]; stats read from fp16)
    rng = ppool.tile([P, wpp], F32, tag="rng")
    nc.vector.scalar_tensor_tensor(
        rng[:], fmaxh[:], 1e-30, fminh[:], AL.add, AL.subtract
    )
    rinv = ppool.tile([P, wpp], F32, tag="rinv")
    nc.vector.reciprocal(rinv[:], rng[:])
    u = ppool.tile([P, wpp], F32, tag="u")
    nc.vector.scalar_tensor_tensor(u[:], lm1[:], 0.0, rinv[:], AL.add, AL.mult)
    c2 = ppool.tile([P, wpp], F32, tag="c2")
    nc.vector.scalar_tensor_tensor(c2[:], u[:], -1.0, fminh[:], AL.mult, AL.mult)
    ilm1 = ppool.tile([P, wpp], F32, tag="ilm1")
    nc.vector.reciprocal(ilm1[:], lm1[:])
    v = ppool.tile([P, wpp], F32, tag="v")
    nc.vector.scalar_tensor_tensor(v[:], rng[:], 0.0, ilm1[:], AL.add, AL.mult)
    vh = ppool.tile([P, wpp], FP16, tag="vh")
    nc.vector.tensor_copy(vh[:], v[:])
    st["u"] = u
    st["c2"] = c2
    st["vh"] = vh
    st["fminh"] = fminh


def _w_pre_round(nc, fpool, st, wpp, fb_bufs):
    """Stage C: *u (Pool) + +c2 (DVE slice + Pool) + ACT round -> fp16 Q."""
    F, Fv = st["F"], st["Fv"]
    u, c2 = st["u"], st["c2"]
    _p1 = nc.gpsimd.tensor_tensor(Fv, Fv, _bcast(u, C, wpp), AL.mult)
    k = PRE2DVE
    if k > 0:
        nc.vector.tensor_tensor(
            Fv[:, :k, :], Fv[:, :k, :], _bcast(c2, k, wpp), AL.add
        )
    if k < C:
        _p2 = nc.gpsimd.tensor_tensor(
            Fv[:, k:, :], Fv[:, k:, :], _bcast(c2, C - k, wpp), AL.add
        )
    nc.scalar.activation(F[:], F[:], ACTF.Copy, bias=M_MAGIC, scale=1.0)
    Q = fpool.tile([P, C * wpp], FP16, tag="Q", bufs=fb_bufs)
    nc.scalar.activation(Q[:], F[:], ACTF.Copy, bias=-M_MAGIC, scale=1.0)
    st["Q"] = Q


def _w_tails_store(nc, st, out, wpp):
    """Stage D: DVE fp16 tails + SP-ring stores."""
    SB_PX = P * wpp
    b, px0 = st["b"], st["px0"]
    Q = st["Q"]
    Qv = Q[:].rearrange("p (c w) -> p c w", c=C)
    nc.vector.tensor_tensor(Qv, Qv, _bcast(st["vh"], C, wpp), AL.mult)
    nc.vector.tensor_tensor(Qv, Qv, _bcast(st["fminh"], C, wpp), AL.add)
    for cc in range(0, C, CCH):
        dst = out[b, cc : cc + CCH, px0 : px0 + SB_PX]
        dst = dst.rearrange("c (p w) -> p c w", p=P)
        nc.sync.dma_start(out=dst, in_=Qv[:, cc : cc + CCH, :])


def _build_full(nc, tc, feat, bits, out, reps, wpp, f_bufs, fb_bufs, n_sb):
    """Staged software pipeline: loads(k) | stats(k-1) | tails(k-3) | pre(k-2)."""
    with (
        tc.tile_pool(name="fpool", bufs=2) as fpool,
        tc.tile_pool(name="ppool", bufs=2) as ppool,
    ):
        for _rep in range(reps):
            blocks = [(b, s) for b in range(B_LOC) for s in range(n_sb)]
            n = len(blocks)
            pend = {}
            for k in range(n + 3):
                if k < n:
                    b, s = blocks[k]
                    pend[k] = _w_loads(nc, fpool, ppool, feat, bits, b, s,
                                       wpp, f_bufs)
                if k >= 1 and k - 1 < n:
                    _w_stats(nc, fpool, ppool, pend[k - 1], wpp)
                if k >= 3:
                    _w_tails_store(nc, pend[k - 3], out, wpp)
                    del pend[k - 3]
                if k >= 2 and k - 2 < n:
                    _w_pre_round(nc, fpool, pend[k - 2], wpp, fb_bufs)


def build(
    reps: int = 1,
    variant: str = "full",
    timed_loop: int = 0,
    wpp: int = None,
    f_bufs: int = None,
) -> bass.Bass:
    """Build the per-core Bass program.

    reps: python-unrolled repetitions of the whole (idempotent) workload.
    variant: full | dma (bisection aid: loads + cast + stores only).
    timed_loop: if >0, build a timing-only program: internal DRAM tensors
    (no input upload), tiny dummy output, and a hardware For_i loop running
    the workload `timed_loop` times.
    """
    _patch_tile_drain()
    if wpp is None:
        wpp = WPP
    if f_bufs is None:
        f_bufs = F_BUFS
    fb_bufs = FB_BUFS
    n_sb = PX // (P * wpp)
    out_dt = FP16 if variant == "full" else BF16
    nc = bass.Bass()
    if timed_loop:
        feat = nc.dram_tensor("features_i", [B_LOC, C, PX], F32)
        bits = nc.dram_tensor("bits_i", [B_LOC, PX], I32)
        out = nc.dram_tensor("out_i", [B_LOC, C, PX], out_dt)
        dummy = nc.declare_dram_parameter("out", [1, 128], F32, isOutput=True)
    else:
        feat = nc.declare_dram_parameter(
            "features", [B_LOC, C, PX], F32, isOutput=False
        )
        bits = nc.declare_dram_parameter(
            "bit_allocation", [B_LOC, PX], I32, isOutput=False
        )
        out = nc.declare_dram_parameter(
            "out", [B_LOC, C, PX], out_dt, isOutput=True
        )

    with TileContext(nc) as tc:
        loop_cm = tc.For_i(0, timed_loop, 1) if timed_loop else nullcontext()
        with loop_cm:
            if variant == "full":
                _build_full(nc, tc, feat, bits, out, reps, wpp, f_bufs,
                            fb_bufs, n_sb)
            else:
                with (
                    tc.tile_pool(name="fpool", bufs=2) as fpool,
                    tc.tile_pool(name="ppool", bufs=2) as ppool,
                ):
                    for _rep in range(reps):
                        blocks = [
                            (b, s) for b in range(B_LOC) for s in range(n_sb)
                        ]
                        pend = {}
                        for i in range(len(blocks) + BACKLAG):
                            if i < len(blocks):
                                b, s = blocks[i]
                                pend[i] = _front_a(
                                    nc, fpool, ppool, feat, bits, b, s,
                                    variant, wpp, f_bufs, fb_bufs,
                                )
                            j = i - BACKLAG
                            if j >= 0:
                                _back(nc, pend[j], out, variant, wpp)
                                del pend[j]
                            if i < len(blocks):
                                _front_b(nc, pend[i], variant, wpp)
        if timed_loop:
            with tc.tile_pool(name="dpool", bufs=1) as dpool:
                dtile = dpool.tile([1, 128], F32, tag="dummy")
                nc.vector.memset(dtile[:], 0.0)
                nc.sync.dma_start(out=dummy[:], in_=dtile[:])
    _split_sync_waits(nc)
    return nc


_nc_cache: dict = {}


def _get_nc(reps: int = 1, variant: str = "full", timed_loop: int = 0) -> bass.Bass:
    key = (reps, variant, timed_loop)
    if key not in _nc_cache:
        _nc_cache[key] = build(reps, variant, timed_loop)
    return _nc_cache[key]


def _in_maps(features: np.ndarray, bit_allocation: np.ndarray):
    f = np.ascontiguousarray(features, dtype=np.float32).reshape(B_FULL, C, PX)
    ba = np.ascontiguousarray(bit_allocation, dtype=np.int32).reshape(B_FULL, PX)
    maps = []
    for i in range(N_CORES):
        b0 = i * B_LOC
        maps.append(
            {
                "features": f[b0 : b0 + B_LOC],
                "bit_allocation": ba[b0 : b0 + B_LOC],
            }
        )
    return maps


def run(
    features: np.ndarray,
    bit_allocation: np.ndarray,
    reps: int = 1,
    variant: str = "full",
):
    nc = _get_nc(reps, variant)
    maps = _in_maps(features, bit_allocation)
    res = run_bass_kernel_spmd(nc, maps, core_ids=list(range(N_CORES)))
    outs = [
        np.asarray(res.results[i]["out"]).astype(np.float32).reshape(
            B_LOC, C, H, W
        )
        for i in range(N_CORES)
    ]
    return np.concatenate(outs, axis=0)


def run_timed(timed_loop: int, variant: str = "full", reps: int = 1):
    """Run the timing-only program (no input upload); returns nothing useful."""
    nc = _get_nc(reps, variant, timed_loop)
    maps = [{} for _ in range(N_CORES)]
    run_bass_kernel_spmd(nc, maps, core_ids=list(range(N_CORES)))


def kernel(features: np.ndarray, bit_allocation: np.ndarray) -> np.ndarray:
    return run(features, bit_allocation, reps=1)


# revision 30
# speedup vs baseline: 1.3548x; 1.1055x over previous
"""AdaptiveQuantizer Trainium2 kernel (8 NeuronCores, data-parallel over batch).

Math (per pixel (b,h,w), over C=64 channels):
    fmin/fmax = min/max over channels
    rng  = (fmax + 1e-30) - fmin
    lm1  = 2**bits - 1                (exact, via int shift trick)
    u    = lm1 / rng ;  c2 = -u*fmin ;  v = rng / lm1
    w    = u*f + c2                   in [0, lm1]
    r    = round_half_even(w)         via fp32 +M / -M (M = 1.5*2**23)
    out  = v*r + fmin                 (bf16 out; host casts to f32)

Engine balance (cost-model driven; all per 1M-elem superblock). Container
walrus constraints: Pool accepts only tensor_tensor add/mult (0.42 eff,
16.3us/pass) and tensor_scalar (0.6 eff); Pool stt / tt-max / free-dim
reduce are rejected, so min/max stats are DVE-only.
  * DVE: strided 64-ch min/max reduces (8.7 each, 1x), params, a small
    channel slice of the +c2 pass, bf16 tails *v and +fmin (4.3 each,
    2x_1p)                                                 -> ~29us
  * Pool: the *u pass (tt-mult-bcast 16.3) and most of the +c2 pass
    (tt-add-bcast ~13)                                     -> ~29us
  * ACT: +M (7.0) and -M -> bf16 (7.0) rounding passes + bf16 param copies
  * DMA: f32 loads on the SP HWDGE ring, bf16 stores on the ACT HWDGE ring
    (separate FIFOs, no head-of-line blocking), ~19-23us
"""

import os
import sys
from contextlib import nullcontext

for _p in ("/opt/trn_rl_repo", "/root/.axon_site/_ro/trn_rl_repo"):
    if os.path.isdir(_p) and _p not in sys.path:
        sys.path.insert(0, _p)

import numpy as np

import concourse.bass as bass
import concourse.mybir as mybir
from concourse.bass_utils import run_bass_kernel_spmd
from concourse.tile import TileContext
from concourse.vector_clock import ScopedClock

# Problem shapes (hardcoded per spec)
B_FULL, C, H, W = 16, 64, 256, 256
N_CORES = 8
B_LOC = B_FULL // N_CORES  # images per core
PX = H * W                 # pixels per image
P = 128                    # SBUF partitions
WPP = int(os.environ.get("KWPP", "128"))   # pixels per partition per superblock
F_BUFS = int(os.environ.get("KFBUFS", "3"))
FB_BUFS = int(os.environ.get("KFBBUFS", "3"))
FH_BUFS = int(os.environ.get("KFHBUFS", "2"))
CCH = int(os.environ.get("KCCH", "16"))    # channels per DMA chunk
PRE2DVE = int(os.environ.get("KPRE2DVE", "32"))  # +c2 channels on DVE (rest Pool)
PREDVE = int(os.environ.get("KPREDVE", "17"))  # channels with BOTH pre passes on DVE
BACKLAG = int(os.environ.get("KBACKLAG", "2"))
KCAST = os.environ.get("KCAST", "act")     # f32->fp16 cast engine: dve | act
KROUND = os.environ.get("KROUND", "act")   # +M/-M rounding: act | dve
KSTORE = os.environ.get("KSTORE", "sp")    # store HWDGE ring: sp | act
M_MAGIC = 12582912.0       # 1.5*2**23: fp32 "+M" add == round-to-nearest-even
AL = mybir.AluOpType
F32 = mybir.dt.float32
I32 = mybir.dt.int32
BF16 = mybir.dt.bfloat16
FP16 = mybir.dt.float16
ACTF = mybir.ActivationFunctionType

_drain_patched = False


def _patch_tile_drain():
    """This container's walrus accepts only ONE sync wait per TPB_CTRL
    instruction; Tile's final drain carries one wait per ticked proc.
    Split them across multiple drains."""
    global _drain_patched
    if _drain_patched:
        return
    _drain_patched = True

    def _patched(self, tick_clock, wait_clock):
        nc = self.nc
        drain_inst = nc.sync.drain()
        wait_clock.add_sem_waits(
            drain_inst.ins, ScopedClock({None: tick_clock.global_clock})
        )
        si = drain_inst.ins.sync_info
        waits = list(si.on_wait) if (si is not None and si.on_wait) else []
        if len(waits) > 1:
            si.on_wait = waits[:1]
            for wchunk in waits[1:]:
                extra = nc.sync.drain()
                esi = extra.ins.sync_info
                if esi is None:
                    extra.ins.sync_info = mybir.SyncInfo(
                        on_wait=[wchunk], on_update=[]
                    )
                else:
                    esi.on_wait = [wchunk]
        nc.all_engine_barrier()
        assert self.sems is not None
        popped = nc._tile_sem_poison_stack.pop()
        assert popped is self._sem_poison
        nc.clear_and_free_semaphores(list(self.sems.allocated().values()))
        nc.all_engine_barrier()

    TileContext._drain_and_barrier = _patched


def _split_sync_waits(nc: bass.Bass, max_waits: int = 1) -> None:
    """This container's walrus rejects instructions carrying more than one
    sync wait. Hoist excess waits onto injected same-engine NOPs placed
    immediately before the instruction (engine program order makes this
    semantically identical)."""
    k = 0
    for bb in nc.main_func.blocks:
        insts = list(bb.instructions)
        out_list = []
        changed = False
        for inst in insts:
            si = inst.sync_info
            waits = list(si.on_wait) if (si is not None and si.on_wait) else []
            if len(waits) > max_waits:
                keep = waits[-max_waits:]
                hoist = waits[:-max_waits]
                for i in range(0, len(hoist), max_waits):
                    nop = mybir.InstNoOp(name=f"WSPL-{k}", ins=[], outs=[])
                    k += 1
                    nop.engine = inst.engine
                    nop.sync_info = mybir.SyncInfo(
                        on_wait=hoist[i : i + max_waits], on_update=[]
                    )
                    out_list.append(nop)
                si.on_wait = keep
                changed = True
            out_list.append(inst)
        if changed:
            bb.instructions.clear()
            for inst in out_list:
                bb.instructions.append(inst)


def _bcast(t, nch, wpp):
    return t[:].rearrange("p (o w) -> p o w", o=1).to_broadcast((P, nch, wpp))


def _vflags(variant):
    """Stage flags per variant (bisection aids)."""
    full = variant == "full"
    return {
        "stats": full or variant in ("red", "tree"),
        "tree": variant == "tree",
        "pre": "split" if full else (
            "dve" if variant == "pre_dve" else
            "pool" if variant == "pre_pool" else None
        ),
        "round": (
            "act" if full or variant in ("act2", "act2_sp") else
            "dve" if variant == "rounddve" else None
        ),
        "tails": full,
        "sp_store": variant in ("dma_sp", "act2_sp"),
    }


def _emit_stats(nc, ppool, Fv, wpp):
    fmax = ppool.tile([P, wpp], F32, tag="fmax")
    fmin = ppool.tile([P, wpp], F32, tag="fmin")
    tmax = ppool.tile([P, wpp], F32, tag="tmax")
    tmin = ppool.tile([P, wpp], F32, tag="tmin")
    h = C // 2
    nc.vector.tensor_reduce(
        fmax[:], Fv[:, :h, :].rearrange("p c w -> p w c"),
        axis=mybir.AxisListType.X, op=AL.max,
    )
    nc.vector.tensor_reduce(
        fmin[:], Fv[:, :h, :].rearrange("p c w -> p w c"),
        axis=mybir.AxisListType.X, op=AL.min,
    )
    nc.vector.tensor_reduce(
        tmax[:], Fv[:, h:, :].rearrange("p c w -> p w c"),
        axis=mybir.AxisListType.X, op=AL.max,
    )
    nc.vector.tensor_reduce(
        tmin[:], Fv[:, h:, :].rearrange("p c w -> p w c"),
        axis=mybir.AxisListType.X, op=AL.min,
    )
    nc.vector.tensor_tensor(fmax[:], fmax[:], tmax[:], AL.max)
    nc.vector.tensor_tensor(fmin[:], fmin[:], tmin[:], AL.min)
    return fmax, fmin


def _emit_stats_tree(nc, ppool, Fv, wpp):
    """Baseline-style contiguous stt tree (64->32->16->8->4, reduce 4)."""
    fmax = ppool.tile([P, wpp], F32, tag="fmax")
    fmin = ppool.tile([P, wpp], F32, tag="fmin")
    sA = ppool.tile([P, (C // 2) * wpp], F32, tag="sA", bufs=1)
    sB = ppool.tile([P, (C // 4) * wpp], F32, tag="sB", bufs=1)
    vA = sA[:].rearrange("p (c w) -> p c w", c=C // 2)
    vB = sB[:].rearrange("p (c w) -> p c w", c=C // 4)
    for out_t, op in ((fmax, AL.max), (fmin, AL.min)):
        cur = Fv
        nch = C
        views = [vA, vB]
        bi = 0
        while nch > 4:
            half = nch // 2
            dst = views[bi % 2][:, :half, :]
            nc.vector.scalar_tensor_tensor(
                dst, cur[:, :half, :], 0.0, cur[:, half:nch, :], AL.add, op
            )
            cur = dst
            nch = half
            bi += 1
        nc.vector.tensor_reduce(
            out_t[:],
            cur[:, 0:4, :].rearrange("p c w -> p w c"),
            axis=mybir.AxisListType.X,
            op=op,
        )
    return fmax, fmin


def _front_a(nc, fpool, ppool, feat, bits, b, s, variant, wpp, f_bufs,
             fb_bufs):
    """Loads + DVE stats + params + ACT bf16 param copies."""
    fl = _vflags(variant)
    SB_PX = P * wpp
    px0 = s * SB_PX
    F = fpool.tile([P, C * wpp], F32, tag="F", bufs=f_bufs)
    Fv = F[:].rearrange("p (c w) -> p c w", c=C)
    # ---- loads (SP HWDGE ring): 1 MiB chunks, contiguous 512 B runs ----
    for cc in range(0, C, CCH):
        src = feat[b, cc : cc + CCH, px0 : px0 + SB_PX]
        src = src.rearrange("c (p w) -> p c w", p=P)
        nc.sync.dma_start(out=Fv[:, cc : cc + CCH, :], in_=src)
    bt = ppool.tile([P, wpp], I32, tag="bt")
    nc.sync.dma_start(
        out=bt[:],
        in_=bits[b, px0 : px0 + SB_PX].rearrange("(p w) -> p w", p=P),
    )

    Fb = fpool.tile([P, C * wpp], BF16, tag="Fb", bufs=fb_bufs)
    st = {"F": F, "Fv": Fv, "Fb": Fb, "b": b, "px0": px0,
          "vb": None, "fminb": None}
    if variant != "full":
        if fl["pre"] == "dve":
            u = ppool.tile([P, wpp], F32, tag="u")
            c2 = ppool.tile([P, wpp], F32, tag="c2")
            nc.vector.memset(u[:], 1.0)
            nc.vector.memset(c2[:], 0.0)
            nc.vector.tensor_tensor(Fv, Fv, _bcast(u, C, wpp), AL.mult)
            nc.vector.tensor_tensor(Fv, Fv, _bcast(c2, C, wpp), AL.add)
        elif fl["pre"] == "pool":
            u = ppool.tile([P, wpp], F32, tag="u")
            c2 = ppool.tile([P, wpp], F32, tag="c2")
            nc.vector.memset(u[:], 1.0)
            nc.vector.memset(c2[:], 0.0)
            _p1 = nc.gpsimd.tensor_tensor(Fv, Fv, _bcast(u, C, wpp), AL.mult)
            _p2 = nc.gpsimd.tensor_tensor(Fv, Fv, _bcast(c2, C, wpp), AL.add)
        if fl["stats"]:
            if fl["tree"]:
                _emit_stats_tree(nc, ppool, Fv, wpp)
            else:
                _emit_stats(nc, ppool, Fv, wpp)
        if fl["round"] == "act":
            nc.scalar.activation(F[:], F[:], ACTF.Copy, bias=M_MAGIC, scale=1.0)
            nc.scalar.activation(Fb[:], F[:], ACTF.Copy, bias=-M_MAGIC, scale=1.0)
        elif fl["round"] == "dve":
            nc.vector.tensor_scalar(
                Fb[:], F[:], M_MAGIC, M_MAGIC, AL.add, AL.subtract
            )
        else:
            nc.vector.tensor_copy(Fb[:], F[:])
        return st

    # ---- channel min/max: two half-channel strided DVE reduces each, so
    # the first starts as soon as the first two DMA chunks land ----
    fmax, fmin = _emit_stats(nc, ppool, Fv, wpp)

    # ---- lm1 = 2**bits - 1 exactly: (bits+127)<<23 bitcast f32, -1 ----
    lvl_i = ppool.tile([P, wpp], I32, tag="lvl_i")
    nc.vector.tensor_scalar_add(lvl_i[:], bt[:], 127)
    nc.vector.tensor_scalar(lvl_i[:], lvl_i[:], 23, None, AL.logical_shift_left)
    lm1 = ppool.tile([P, wpp], F32, tag="lm1")
    nc.vector.tensor_scalar_add(lm1[:], lvl_i[:].bitcast(F32), -1.0)

    # ---- per-pixel params ([P, wpp] tiles, small DVE ops) ----
    rng = ppool.tile([P, wpp], F32, tag="rng")
    nc.vector.scalar_tensor_tensor(
        rng[:], fmax[:], 1e-30, fmin[:], AL.add, AL.subtract
    )
    rinv = ppool.tile([P, wpp], F32, tag="rinv")
    nc.vector.reciprocal(rinv[:], rng[:])
    u = ppool.tile([P, wpp], F32, tag="u")
    nc.vector.scalar_tensor_tensor(u[:], lm1[:], 0.0, rinv[:], AL.add, AL.mult)
    c2 = ppool.tile([P, wpp], F32, tag="c2")
    nc.vector.scalar_tensor_tensor(c2[:], u[:], -1.0, fmin[:], AL.mult, AL.mult)
    ilm1 = ppool.tile([P, wpp], F32, tag="ilm1")
    nc.vector.reciprocal(ilm1[:], lm1[:])
    v = ppool.tile([P, wpp], F32, tag="v")
    nc.vector.scalar_tensor_tensor(v[:], rng[:], 0.0, ilm1[:], AL.add, AL.mult)
    # bf16 copies of v / fmin for the tails (ACT)
    vb = ppool.tile([P, wpp], BF16, tag="vb")
    nc.scalar.activation(vb[:], v[:], ACTF.Copy, bias=0.0, scale=1.0)
    fminb = ppool.tile([P, wpp], BF16, tag="fminb")
    nc.scalar.activation(fminb[:], fmin[:], ACTF.Copy, bias=0.0, scale=1.0)
    st["u"] = u
    st["c2"] = c2
    st["vb"] = vb
    st["fminb"] = fminb
    return st


def _front_b(nc, st, variant, wpp):
    """Pre-round passes (*u on Pool, +c2 split DVE/Pool) + ACT rounding."""
    if variant != "full":
        return
    F, Fv, Fb = st["F"], st["Fv"], st["Fb"]
    u, c2 = st["u"], st["c2"]
    # *u: Pool tt-mult with broadcast (walrus allows Pool tt add/mult only)
    _p1 = nc.gpsimd.tensor_tensor(Fv, Fv, _bcast(u, C, wpp), AL.mult)
    # +c2: small DVE slice + Pool for the rest, disjoint channel ranges
    k = PRE2DVE
    if k > 0:
        nc.vector.tensor_tensor(
            Fv[:, :k, :], Fv[:, :k, :], _bcast(c2, k, wpp), AL.add
        )
    if k < C:
        _p2 = nc.gpsimd.tensor_tensor(
            Fv[:, k:, :], Fv[:, k:, :], _bcast(c2, C - k, wpp), AL.add
        )
    # ---- rounding on ACT: +M (f32, in place), then -M -> bf16 (exact) ----
    nc.scalar.activation(F[:], F[:], ACTF.Copy, bias=M_MAGIC, scale=1.0)
    nc.scalar.activation(Fb[:], F[:], ACTF.Copy, bias=-M_MAGIC, scale=1.0)


def _back(nc, st, out, variant, wpp):
    """Post-round bf16 tails (DVE 2x tt) + bf16 store on the ACT HWDGE ring."""
    SB_PX = P * wpp
    b, px0 = st["b"], st["px0"]
    Fb = st["Fb"]
    Fbv = Fb[:].rearrange("p (c w) -> p c w", c=C)
    if st["vb"] is not None:
        nc.vector.tensor_tensor(Fbv, Fbv, _bcast(st["vb"], C, wpp), AL.mult)
        nc.vector.tensor_tensor(Fbv, Fbv, _bcast(st["fminb"], C, wpp), AL.add)
    eng = nc.sync if _vflags(variant)["sp_store"] else nc.scalar
    for cc in range(0, C, CCH):
        dst = out[b, cc : cc + CCH, px0 : px0 + SB_PX]
        dst = dst.rearrange("c (p w) -> p c w", p=P)
        eng.dma_start(out=dst, in_=Fbv[:, cc : cc + CCH, :])


def _w_loads(nc, fpool, ppool, feat, bits, b, s, wpp, f_bufs):
    """Stage A: SP-ring loads for superblock (b, s)."""
    SB_PX = P * wpp
    px0 = s * SB_PX
    F = fpool.tile([P, C * wpp], F32, tag="F", bufs=f_bufs)
    Fv = F[:].rearrange("p (c w) -> p c w", c=C)
    for cc in range(0, C, CCH):
        src = feat[b, cc : cc + CCH, px0 : px0 + SB_PX]
        src = src.rearrange("c (p w) -> p c w", p=P)
        nc.sync.dma_start(out=Fv[:, cc : cc + CCH, :], in_=src)
    bt = ppool.tile([P, wpp], I32, tag="bt")
    nc.sync.dma_start(
        out=bt[:],
        in_=bits[b, px0 : px0 + SB_PX].rearrange("(p w) -> p w", p=P),
    )
    return {"F": F, "Fv": Fv, "bt": bt, "b": b, "px0": px0}


def _w_stats(nc, fpool, ppool, st, wpp):
    """Stage B: f32->fp16 cast + DVE fp16 min/max tree + params."""
    F, Fv, bt = st["F"], st["Fv"], st["bt"]
    Fh = fpool.tile([P, C * wpp], FP16, tag="Fh", bufs=FH_BUFS)
    if KCAST == "act":
        nc.scalar.activation(Fh[:], F[:], ACTF.Copy, bias=0.0, scale=1.0)
    else:
        nc.vector.tensor_copy(Fh[:], F[:])
    Fhv = Fh[:].rearrange("p (c w) -> p c w", c=C)
    fmaxh = ppool.tile([P, wpp], FP16, tag="fmaxh")
    fminh = ppool.tile([P, wpp], FP16, tag="fminh")
    sA = ppool.tile([P, (C // 2) * wpp], FP16, tag="sA", bufs=1)
    sB = ppool.tile([P, (C // 4) * wpp], FP16, tag="sB", bufs=1)
    vA = sA[:].rearrange("p (c w) -> p c w", c=C // 2)
    vB = sB[:].rearrange("p (c w) -> p c w", c=C // 4)
    for out_t, op in ((fmaxh, AL.max), (fminh, AL.min)):
        cur = Fhv
        nch = C
        views = [vA, vB]
        bi = 0
        while nch > 4:
            half = nch // 2
            dst = views[bi % 2][:, :half, :]
            nc.vector.tensor_tensor(dst, cur[:, :half, :], cur[:, half:nch, :], op)
            cur = dst
            nch = half
            bi += 1
        nc.vector.tensor_reduce(
            out_t[:], cur[:, 0:4, :].rearrange("p c w -> p w c"),
            axis=mybir.AxisListType.X, op=op,
        )
    # lm1 = 2**bits - 1 exactly
    lvl_i = ppool.tile([P, wpp], I32, tag="lvl_i")
    nc.vector.tensor_scalar_add(lvl_i[:], bt[:], 127)
    nc.vector.tensor_scalar(lvl_i[:], lvl_i[:], 23, None, AL.logical_shift_left)
    lm1 = ppool.tile([P, wpp], F32, tag="lm1")
    nc.vector.tensor_scalar_add(lm1[:], lvl_i[:].bitcast(F32), -1.0)
    # per-pixel params (f32 [P, wpp]; stats read from fp16)
    rng = ppool.tile([P, wpp], F32, tag="rng")
    nc.vector.scalar_tensor_tensor(
        rng[:], fmaxh[:], 1e-30, fminh[:], AL.add, AL.subtract
    )
    rinv = ppool.tile([P, wpp], F32, tag="rinv")
    nc.vector.reciprocal(rinv[:], rng[:])
    u = ppool.tile([P, wpp], F32, tag="u")
    nc.vector.scalar_tensor_tensor(u[:], lm1[:], 0.0, rinv[:], AL.add, AL.mult)
    c2 = ppool.tile([P, wpp], F32, tag="c2")
    nc.vector.scalar_tensor_tensor(c2[:], u[:], -1.0, fminh[:], AL.mult, AL.mult)
    ilm1 = ppool.tile([P, wpp], F32, tag="ilm1")
    nc.vector.reciprocal(ilm1[:], lm1[:])
    v = ppool.tile([P, wpp], F32, tag="v")
    nc.vector.scalar_tensor_tensor(v[:], rng[:], 0.0, ilm1[:], AL.add, AL.mult)
    vh = ppool.tile([P, wpp], FP16, tag="vh")
    nc.vector.tensor_copy(vh[:], v[:])
    st["u"] = u
    st["c2"] = c2
    st["vh"] = vh
    st["fminh"] = fminh


def _w_pre_round(nc, fpool, st, wpp, fb_bufs):
    """Stage C: per-channel-range *u, +c2 (DVE range || Pool range) + round."""
    F, Fv = st["F"], st["Fv"]
    u, c2 = st["u"], st["c2"]
    k = PREDVE
    # DVE range: both passes on channels [0:k); Pool range: [k:C). The two
    # ranges are independent so the engines run their chains in parallel.
    if k > 0:
        nc.vector.tensor_tensor(
            Fv[:, :k, :], Fv[:, :k, :], _bcast(u, k, wpp), AL.mult
        )
        nc.vector.tensor_tensor(
            Fv[:, :k, :], Fv[:, :k, :], _bcast(c2, k, wpp), AL.add
        )
    if k < C:
        _p1 = nc.gpsimd.tensor_tensor(
            Fv[:, k:, :], Fv[:, k:, :], _bcast(u, C - k, wpp), AL.mult
        )
        _p2 = nc.gpsimd.tensor_tensor(
            Fv[:, k:, :], Fv[:, k:, :], _bcast(c2, C - k, wpp), AL.add
        )
    Q = fpool.tile([P, C * wpp], FP16, tag="Q", bufs=fb_bufs)
    if KROUND == "act":
        nc.scalar.activation(F[:], F[:], ACTF.Copy, bias=M_MAGIC, scale=1.0)
        nc.scalar.activation(Q[:], F[:], ACTF.Copy, bias=-M_MAGIC, scale=1.0)
    else:
        nc.vector.tensor_scalar(Q[:], F[:], M_MAGIC, M_MAGIC, AL.add, AL.subtract)
    st["Q"] = Q


def _w_tails_store(nc, st, out, wpp):
    """Stage D: DVE fp16 tails + SP-ring stores."""
    SB_PX = P * wpp
    b, px0 = st["b"], st["px0"]
    Q = st["Q"]
    Qv = Q[:].rearrange("p (c w) -> p c w", c=C)
    nc.vector.tensor_tensor(Qv, Qv, _bcast(st["vh"], C, wpp), AL.mult)
    nc.vector.tensor_tensor(Qv, Qv, _bcast(st["fminh"], C, wpp), AL.add)
    eng = nc.scalar if KSTORE == "act" else nc.sync
    for cc in range(0, C, CCH):
        dst = out[b, cc : cc + CCH, px0 : px0 + SB_PX]
        dst = dst.rearrange("c (p w) -> p c w", p=P)
        eng.dma_start(out=dst, in_=Qv[:, cc : cc + CCH, :])


def _build_full(nc, tc, feat, bits, out, reps, wpp, f_bufs, fb_bufs, n_sb):
    """Staged software pipeline: loads(k) | stats(k-1) | tails(k-3) | pre(k-2)."""
    with (
        tc.tile_pool(name="fpool", bufs=2) as fpool,
        tc.tile_pool(name="ppool", bufs=2) as ppool,
    ):
        for _rep in range(reps):
            blocks = [(b, s) for b in range(B_LOC) for s in range(n_sb)]
            n = len(blocks)
            pend = {}
            for k in range(n + 3):
                if k < n:
                    b, s = blocks[k]
                    pend[k] = _w_loads(nc, fpool, ppool, feat, bits, b, s,
                                       wpp, f_bufs)
                if k >= 1 and k - 1 < n:
                    _w_stats(nc, fpool, ppool, pend[k - 1], wpp)
                if k >= 3:
                    _w_tails_store(nc, pend[k - 3], out, wpp)
                    del pend[k - 3]
                if k >= 2 and k - 2 < n:
                    _w_pre_round(nc, fpool, pend[k - 2], wpp, fb_bufs)


def build(
    reps: int = 1,
    variant: str = "full",
    timed_loop: int = 0,
    wpp: int = None,
    f_bufs: int = None,
) -> bass.Bass:
    """Build the per-core Bass program.

    reps: python-unrolled repetitions of the whole (idempotent) workload.
    variant: full | dma (bisection aid: loads + cast + stores only).
    timed_loop: if >0, build a timing-only program: internal DRAM tensors
    (no input upload), tiny dummy output, and a hardware For_i loop running
    the workload `timed_loop` times.
    """
    _patch_tile_drain()
    if wpp is None:
        wpp = WPP
    if f_bufs is None:
        f_bufs = F_BUFS
    fb_bufs = FB_BUFS
    n_sb = PX // (P * wpp)
    out_dt = FP16 if variant == "full" else BF16
    nc = bass.Bass()
    if timed_loop:
        feat = nc.dram_tensor("features_i", [B_LOC, C, PX], F32)
        bits = nc.dram_tensor("bits_i", [B_LOC, PX], I32)
        out = nc.dram_tensor("out_i", [B_LOC, C, PX], out_dt)
        dummy = nc.declare_dram_parameter("out", [1, 128], F32, isOutput=True)
    else:
        feat = nc.declare_dram_parameter(
            "features", [B_LOC, C, PX], F32, isOutput=False
        )
        bits = nc.declare_dram_parameter(
            "bit_allocation", [B_LOC, PX], I32, isOutput=False
        )
        out = nc.declare_dram_parameter(
            "out", [B_LOC, C, PX], out_dt, isOutput=True
        )

    with TileContext(nc) as tc:
        loop_cm = tc.For_i(0, timed_loop, 1) if timed_loop else nullcontext()
        with loop_cm:
            if variant == "full":
                _build_full(nc, tc, feat, bits, out, reps, wpp, f_bufs,
                            fb_bufs, n_sb)
            else:
                with (
                    tc.tile_pool(name="fpool", bufs=2) as fpool,
                    tc.tile_pool(name="ppool", bufs=2) as ppool,
                ):
                    for _rep in range(reps):
                        blocks = [
                            (b, s) for b in range(B_LOC) for s in range(n_sb)
                        ]
                        pend = {}
                        for i in range(len(blocks) + BACKLAG):
                            if i < len(blocks):
                                b, s = blocks[i]
                                pend[i] = _front_a(
                                    nc, fpool, ppool, feat, bits, b, s,
                                    variant, wpp, f_bufs, fb_bufs,
                                )
                            j = i - BACKLAG
                            if j >= 0:
                                _back(nc, pend[j], out, variant, wpp)
                                del pend[j]
                            if i < len(blocks):
                                _front_b(nc, pend[i], variant, wpp)
        if timed_loop:
            with tc.tile_pool(name="dpool", bufs=1) as dpool:
                dtile = dpool.tile([1, 128], F32, tag="dummy")
                nc.vector.memset(dtile[:], 0.0)
                nc.sync.dma_start(out=dummy[:], in_=dtile[:])
    _split_sync_waits(nc)
    return nc


_nc_cache: dict = {}


def _get_nc(reps: int = 1, variant: str = "full", timed_loop: int = 0) -> bass.Bass:
    key = (reps, variant, timed_loop)
    if key not in _nc_cache:
        _nc_cache[key] = build(reps, variant, timed_loop)
    return _nc_cache[key]


def _in_maps(features: np.ndarray, bit_allocation: np.ndarray):
    f = np.ascontiguousarray(features, dtype=np.float32).reshape(B_FULL, C, PX)
    ba = np.ascontiguousarray(bit_allocation, dtype=np.int32).reshape(B_FULL, PX)
    maps = []
    for i in range(N_CORES):
        b0 = i * B_LOC
        maps.append(
            {
                "features": f[b0 : b0 + B_LOC],
                "bit_allocation": ba[b0 : b0 + B_LOC],
            }
        )
    return maps


def run(
    features: np.ndarray,
    bit_allocation: np.ndarray,
    reps: int = 1,
    variant: str = "full",
):
    nc = _get_nc(reps, variant)
    maps = _in_maps(features, bit_allocation)
    res = run_bass_kernel_spmd(nc, maps, core_ids=list(range(N_CORES)))
    outs = [
        np.asarray(res.results[i]["out"]).astype(np.float32).reshape(
            B_LOC, C, H, W
        )
        for i in range(N_CORES)
    ]
    return np.concatenate(outs, axis=0)


def run_timed(timed_loop: int, variant: str = "full", reps: int = 1):
    """Run the timing-only program (no input upload); returns nothing useful."""
    nc = _get_nc(reps, variant, timed_loop)
    maps = [{} for _ in range(N_CORES)]
    run_bass_kernel_spmd(nc, maps, core_ids=list(range(N_CORES)))


def kernel(features: np.ndarray, bit_allocation: np.ndarray) -> np.ndarray:
    return run(features, bit_allocation, reps=1)


# revision 31
# speedup vs baseline: 1.3902x; 1.0261x over previous
"""AdaptiveQuantizer Trainium2 kernel (8 NeuronCores, data-parallel over batch).

Math (per pixel (b,h,w), over C=64 channels):
    fmin/fmax = min/max over channels
    rng  = (fmax + 1e-30) - fmin
    lm1  = 2**bits - 1                (exact, via int shift trick)
    u    = lm1 / rng ;  c2 = -u*fmin ;  v = rng / lm1
    w    = u*f + c2                   in [0, lm1]
    r    = round_half_even(w)         via fp32 +M / -M (M = 1.5*2**23)
    out  = v*r + fmin                 (bf16 out; host casts to f32)

Engine assignment (HW-bisection driven; per 1M-elem superblock, 8/core):
  * ACT: f32->fp16 cast of F, then the +M / -M(->fp16) rounding passes.
    (HW-measured: ACT compute is fast ONLY if no HWDGE stores share the
    ACT ring -- mixing them measured ~12x slow. Stores go on the SP ring.)
  * DVE: fp16 min/max tree (tt-max/min 2x_1p packed) + final 4-ch strided
    reduce; per-pixel params; both pre passes on channels [0:PREDVE);
    fp16 tails *v and +fmin (tt broadcast 2x_1p).
  * Pool: both pre passes (*u, +c2) on channels [PREDVE:64) as
    tensor_tensor broadcast (the only Pool ops walrus accepts).
  * Stats/tails/output in fp16: rel L2 err 1.642e-02 (< 2e-2 gate); exact
    f32 stats would cost ~2x stat time (DVE strided/2-stream ops measured
    ~1.6-2x the nominal 1x model).
  * Stage pipeline with one-iteration skew per stage:
    loads(k) | cast+stats(k-1) | pre+round(k-2) | tails+store(k-3).
"""

import os
import sys
from contextlib import nullcontext

for _p in ("/opt/trn_rl_repo", "/root/.axon_site/_ro/trn_rl_repo"):
    if os.path.isdir(_p) and _p not in sys.path:
        sys.path.insert(0, _p)

import numpy as np

import concourse.bass as bass
import concourse.mybir as mybir
from concourse.bass_utils import run_bass_kernel_spmd
from concourse.tile import TileContext
from concourse.vector_clock import ScopedClock

# Problem shapes (hardcoded per spec)
B_FULL, C, H, W = 16, 64, 256, 256
N_CORES = 8
B_LOC = B_FULL // N_CORES  # images per core
PX = H * W                 # pixels per image
P = 128                    # SBUF partitions
WPP = int(os.environ.get("KWPP", "128"))   # pixels per partition per superblock
F_BUFS = int(os.environ.get("KFBUFS", "3"))
FB_BUFS = int(os.environ.get("KFBBUFS", "3"))
FH_BUFS = int(os.environ.get("KFHBUFS", "2"))
CCH = int(os.environ.get("KCCH", "16"))    # channels per DMA chunk
PRE2DVE = int(os.environ.get("KPRE2DVE", "32"))  # +c2 channels on DVE (rest Pool)
PREDVE = int(os.environ.get("KPREDVE", "17"))  # channels with BOTH pre passes on DVE
BACKLAG = int(os.environ.get("KBACKLAG", "2"))
KCAST = os.environ.get("KCAST", "act")     # f32->fp16 cast engine: dve | act
KROUND = os.environ.get("KROUND", "act")   # +M/-M rounding: act | dve
KSTORE = os.environ.get("KSTORE", "sp")    # store HWDGE ring: sp | act
M_MAGIC = 12582912.0       # 1.5*2**23: fp32 "+M" add == round-to-nearest-even
AL = mybir.AluOpType
F32 = mybir.dt.float32
I32 = mybir.dt.int32
BF16 = mybir.dt.bfloat16
FP16 = mybir.dt.float16
ACTF = mybir.ActivationFunctionType

_drain_patched = False


def _patch_tile_drain():
    """This container's walrus accepts only ONE sync wait per TPB_CTRL
    instruction; Tile's final drain carries one wait per ticked proc.
    Split them across multiple drains."""
    global _drain_patched
    if _drain_patched:
        return
    _drain_patched = True

    def _patched(self, tick_clock, wait_clock):
        nc = self.nc
        drain_inst = nc.sync.drain()
        wait_clock.add_sem_waits(
            drain_inst.ins, ScopedClock({None: tick_clock.global_clock})
        )
        si = drain_inst.ins.sync_info
        waits = list(si.on_wait) if (si is not None and si.on_wait) else []
        if len(waits) > 1:
            si.on_wait = waits[:1]
            for wchunk in waits[1:]:
                extra = nc.sync.drain()
                esi = extra.ins.sync_info
                if esi is None:
                    extra.ins.sync_info = mybir.SyncInfo(
                        on_wait=[wchunk], on_update=[]
                    )
                else:
                    esi.on_wait = [wchunk]
        nc.all_engine_barrier()
        assert self.sems is not None
        popped = nc._tile_sem_poison_stack.pop()
        assert popped is self._sem_poison
        nc.clear_and_free_semaphores(list(self.sems.allocated().values()))
        nc.all_engine_barrier()

    TileContext._drain_and_barrier = _patched


def _split_sync_waits(nc: bass.Bass, max_waits: int = 1) -> None:
    """This container's walrus rejects instructions carrying more than one
    sync wait. Hoist excess waits onto injected same-engine NOPs placed
    immediately before the instruction (engine program order makes this
    semantically identical)."""
    k = 0
    for bb in nc.main_func.blocks:
        insts = list(bb.instructions)
        out_list = []
        changed = False
        for inst in insts:
            si = inst.sync_info
            waits = list(si.on_wait) if (si is not None and si.on_wait) else []
            if len(waits) > max_waits:
                keep = waits[-max_waits:]
                hoist = waits[:-max_waits]
                for i in range(0, len(hoist), max_waits):
                    nop = mybir.InstNoOp(name=f"WSPL-{k}", ins=[], outs=[])
                    k += 1
                    nop.engine = inst.engine
                    nop.sync_info = mybir.SyncInfo(
                        on_wait=hoist[i : i + max_waits], on_update=[]
                    )
                    out_list.append(nop)
                si.on_wait = keep
                changed = True
            out_list.append(inst)
        if changed:
            bb.instructions.clear()
            for inst in out_list:
                bb.instructions.append(inst)


def _bcast(t, nch, wpp):
    return t[:].rearrange("p (o w) -> p o w", o=1).to_broadcast((P, nch, wpp))


def _vflags(variant):
    """Stage flags per variant (bisection aids)."""
    full = variant == "full"
    return {
        "stats": full or variant in ("red", "tree"),
        "tree": variant == "tree",
        "pre": "split" if full else (
            "dve" if variant == "pre_dve" else
            "pool" if variant == "pre_pool" else None
        ),
        "round": (
            "act" if full or variant in ("act2", "act2_sp") else
            "dve" if variant == "rounddve" else None
        ),
        "tails": full,
        "sp_store": variant in ("dma_sp", "act2_sp"),
    }


def _emit_stats(nc, ppool, Fv, wpp):
    fmax = ppool.tile([P, wpp], F32, tag="fmax")
    fmin = ppool.tile([P, wpp], F32, tag="fmin")
    tmax = ppool.tile([P, wpp], F32, tag="tmax")
    tmin = ppool.tile([P, wpp], F32, tag="tmin")
    h = C // 2
    nc.vector.tensor_reduce(
        fmax[:], Fv[:, :h, :].rearrange("p c w -> p w c"),
        axis=mybir.AxisListType.X, op=AL.max,
    )
    nc.vector.tensor_reduce(
        fmin[:], Fv[:, :h, :].rearrange("p c w -> p w c"),
        axis=mybir.AxisListType.X, op=AL.min,
    )
    nc.vector.tensor_reduce(
        tmax[:], Fv[:, h:, :].rearrange("p c w -> p w c"),
        axis=mybir.AxisListType.X, op=AL.max,
    )
    nc.vector.tensor_reduce(
        tmin[:], Fv[:, h:, :].rearrange("p c w -> p w c"),
        axis=mybir.AxisListType.X, op=AL.min,
    )
    nc.vector.tensor_tensor(fmax[:], fmax[:], tmax[:], AL.max)
    nc.vector.tensor_tensor(fmin[:], fmin[:], tmin[:], AL.min)
    return fmax, fmin


def _emit_stats_tree(nc, ppool, Fv, wpp):
    """Baseline-style contiguous stt tree (64->32->16->8->4, reduce 4)."""
    fmax = ppool.tile([P, wpp], F32, tag="fmax")
    fmin = ppool.tile([P, wpp], F32, tag="fmin")
    sA = ppool.tile([P, (C // 2) * wpp], F32, tag="sA", bufs=1)
    sB = ppool.tile([P, (C // 4) * wpp], F32, tag="sB", bufs=1)
    vA = sA[:].rearrange("p (c w) -> p c w", c=C // 2)
    vB = sB[:].rearrange("p (c w) -> p c w", c=C // 4)
    for out_t, op in ((fmax, AL.max), (fmin, AL.min)):
        cur = Fv
        nch = C
        views = [vA, vB]
        bi = 0
        while nch > 4:
            half = nch // 2
            dst = views[bi % 2][:, :half, :]
            nc.vector.scalar_tensor_tensor(
                dst, cur[:, :half, :], 0.0, cur[:, half:nch, :], AL.add, op
            )
            cur = dst
            nch = half
            bi += 1
        nc.vector.tensor_reduce(
            out_t[:],
            cur[:, 0:4, :].rearrange("p c w -> p w c"),
            axis=mybir.AxisListType.X,
            op=op,
        )
    return fmax, fmin


def _front_a(nc, fpool, ppool, feat, bits, b, s, variant, wpp, f_bufs,
             fb_bufs):
    """Loads + DVE stats + params + ACT bf16 param copies."""
    fl = _vflags(variant)
    SB_PX = P * wpp
    px0 = s * SB_PX
    F = fpool.tile([P, C * wpp], F32, tag="F", bufs=f_bufs)
    Fv = F[:].rearrange("p (c w) -> p c w", c=C)
    # ---- loads (SP HWDGE ring): 1 MiB chunks, contiguous 512 B runs ----
    for cc in range(0, C, CCH):
        src = feat[b, cc : cc + CCH, px0 : px0 + SB_PX]
        src = src.rearrange("c (p w) -> p c w", p=P)
        nc.sync.dma_start(out=Fv[:, cc : cc + CCH, :], in_=src)
    bt = ppool.tile([P, wpp], I32, tag="bt")
    nc.sync.dma_start(
        out=bt[:],
        in_=bits[b, px0 : px0 + SB_PX].rearrange("(p w) -> p w", p=P),
    )

    Fb = fpool.tile([P, C * wpp], BF16, tag="Fb", bufs=fb_bufs)
    st = {"F": F, "Fv": Fv, "Fb": Fb, "b": b, "px0": px0,
          "vb": None, "fminb": None}
    if variant != "full":
        if fl["pre"] == "dve":
            u = ppool.tile([P, wpp], F32, tag="u")
            c2 = ppool.tile([P, wpp], F32, tag="c2")
            nc.vector.memset(u[:], 1.0)
            nc.vector.memset(c2[:], 0.0)
            nc.vector.tensor_tensor(Fv, Fv, _bcast(u, C, wpp), AL.mult)
            nc.vector.tensor_tensor(Fv, Fv, _bcast(c2, C, wpp), AL.add)
        elif fl["pre"] == "pool":
            u = ppool.tile([P, wpp], F32, tag="u")
            c2 = ppool.tile([P, wpp], F32, tag="c2")
            nc.vector.memset(u[:], 1.0)
            nc.vector.memset(c2[:], 0.0)
            _p1 = nc.gpsimd.tensor_tensor(Fv, Fv, _bcast(u, C, wpp), AL.mult)
            _p2 = nc.gpsimd.tensor_tensor(Fv, Fv, _bcast(c2, C, wpp), AL.add)
        if fl["stats"]:
            if fl["tree"]:
                _emit_stats_tree(nc, ppool, Fv, wpp)
            else:
                _emit_stats(nc, ppool, Fv, wpp)
        if fl["round"] == "act":
            nc.scalar.activation(F[:], F[:], ACTF.Copy, bias=M_MAGIC, scale=1.0)
            nc.scalar.activation(Fb[:], F[:], ACTF.Copy, bias=-M_MAGIC, scale=1.0)
        elif fl["round"] == "dve":
            nc.vector.tensor_scalar(
                Fb[:], F[:], M_MAGIC, M_MAGIC, AL.add, AL.subtract
            )
        else:
            nc.vector.tensor_copy(Fb[:], F[:])
        return st

    # ---- channel min/max: two half-channel strided DVE reduces each, so
    # the first starts as soon as the first two DMA chunks land ----
    fmax, fmin = _emit_stats(nc, ppool, Fv, wpp)

    # ---- lm1 = 2**bits - 1 exactly: (bits+127)<<23 bitcast f32, -1 ----
    lvl_i = ppool.tile([P, wpp], I32, tag="lvl_i")
    nc.vector.tensor_scalar_add(lvl_i[:], bt[:], 127)
    nc.vector.tensor_scalar(lvl_i[:], lvl_i[:], 23, None, AL.logical_shift_left)
    lm1 = ppool.tile([P, wpp], F32, tag="lm1")
    nc.vector.tensor_scalar_add(lm1[:], lvl_i[:].bitcast(F32), -1.0)

    # ---- per-pixel params ([P, wpp] tiles, small DVE ops) ----
    rng = ppool.tile([P, wpp], F32, tag="rng")
    nc.vector.scalar_tensor_tensor(
        rng[:], fmax[:], 1e-30, fmin[:], AL.add, AL.subtract
    )
    rinv = ppool.tile([P, wpp], F32, tag="rinv")
    nc.vector.reciprocal(rinv[:], rng[:])
    u = ppool.tile([P, wpp], F32, tag="u")
    nc.vector.scalar_tensor_tensor(u[:], lm1[:], 0.0, rinv[:], AL.add, AL.mult)
    c2 = ppool.tile([P, wpp], F32, tag="c2")
    nc.vector.scalar_tensor_tensor(c2[:], u[:], -1.0, fmin[:], AL.mult, AL.mult)
    ilm1 = ppool.tile([P, wpp], F32, tag="ilm1")
    nc.vector.reciprocal(ilm1[:], lm1[:])
    v = ppool.tile([P, wpp], F32, tag="v")
    nc.vector.scalar_tensor_tensor(v[:], rng[:], 0.0, ilm1[:], AL.add, AL.mult)
    # bf16 copies of v / fmin for the tails (ACT)
    vb = ppool.tile([P, wpp], BF16, tag="vb")
    nc.scalar.activation(vb[:], v[:], ACTF.Copy, bias=0.0, scale=1.0)
    fminb = ppool.tile([P, wpp], BF16, tag="fminb")
    nc.scalar.activation(fminb[:], fmin[:], ACTF.Copy, bias=0.0, scale=1.0)
    st["u"] = u
    st["c2"] = c2
    st["vb"] = vb
    st["fminb"] = fminb
    return st


def _front_b(nc, st, variant, wpp):
    """Pre-round passes (*u on Pool, +c2 split DVE/Pool) + ACT rounding."""
    if variant != "full":
        return
    F, Fv, Fb = st["F"], st["Fv"], st["Fb"]
    u, c2 = st["u"], st["c2"]
    # *u: Pool tt-mult with broadcast (walrus allows Pool tt add/mult only)
    _p1 = nc.gpsimd.tensor_tensor(Fv, Fv, _bcast(u, C, wpp), AL.mult)
    # +c2: small DVE slice + Pool for the rest, disjoint channel ranges
    k = PRE2DVE
    if k > 0:
        nc.vector.tensor_tensor(
            Fv[:, :k, :], Fv[:, :k, :], _bcast(c2, k, wpp), AL.add
        )
    if k < C:
        _p2 = nc.gpsimd.tensor_tensor(
            Fv[:, k:, :], Fv[:, k:, :], _bcast(c2, C - k, wpp), AL.add
        )
    # ---- rounding on ACT: +M (f32, in place), then -M -> bf16 (exact) ----
    nc.scalar.activation(F[:], F[:], ACTF.Copy, bias=M_MAGIC, scale=1.0)
    nc.scalar.activation(Fb[:], F[:], ACTF.Copy, bias=-M_MAGIC, scale=1.0)


def _back(nc, st, out, variant, wpp):
    """Post-round bf16 tails (DVE 2x tt) + bf16 store on the ACT HWDGE ring."""
    SB_PX = P * wpp
    b, px0 = st["b"], st["px0"]
    Fb = st["Fb"]
    Fbv = Fb[:].rearrange("p (c w) -> p c w", c=C)
    if st["vb"] is not None:
        nc.vector.tensor_tensor(Fbv, Fbv, _bcast(st["vb"], C, wpp), AL.mult)
        nc.vector.tensor_tensor(Fbv, Fbv, _bcast(st["fminb"], C, wpp), AL.add)
    eng = nc.sync if _vflags(variant)["sp_store"] else nc.scalar
    for cc in range(0, C, CCH):
        dst = out[b, cc : cc + CCH, px0 : px0 + SB_PX]
        dst = dst.rearrange("c (p w) -> p c w", p=P)
        eng.dma_start(out=dst, in_=Fbv[:, cc : cc + CCH, :])


def _w_loads(nc, fpool, ppool, feat, bits, b, s, wpp, f_bufs):
    """Stage A: SP-ring loads for superblock (b, s)."""
    SB_PX = P * wpp
    px0 = s * SB_PX
    F = fpool.tile([P, C * wpp], F32, tag="F", bufs=f_bufs)
    Fv = F[:].rearrange("p (c w) -> p c w", c=C)
    for cc in range(0, C, CCH):
        src = feat[b, cc : cc + CCH, px0 : px0 + SB_PX]
        src = src.rearrange("c (p w) -> p c w", p=P)
        nc.sync.dma_start(out=Fv[:, cc : cc + CCH, :], in_=src)
    bt = ppool.tile([P, wpp], I32, tag="bt")
    nc.sync.dma_start(
        out=bt[:],
        in_=bits[b, px0 : px0 + SB_PX].rearrange("(p w) -> p w", p=P),
    )
    return {"F": F, "Fv": Fv, "bt": bt, "b": b, "px0": px0}


def _w_stats(nc, fpool, ppool, st, wpp):
    """Stage B: f32->fp16 cast + DVE fp16 min/max tree + params."""
    F, Fv, bt = st["F"], st["Fv"], st["bt"]
    Fh = fpool.tile([P, C * wpp], FP16, tag="Fh", bufs=FH_BUFS)
    if KCAST == "act":
        nc.scalar.activation(Fh[:], F[:], ACTF.Copy, bias=0.0, scale=1.0)
    else:
        nc.vector.tensor_copy(Fh[:], F[:])
    Fhv = Fh[:].rearrange("p (c w) -> p c w", c=C)
    fmaxh = ppool.tile([P, wpp], FP16, tag="fmaxh")
    fminh = ppool.tile([P, wpp], FP16, tag="fminh")
    sA = ppool.tile([P, (C // 2) * wpp], FP16, tag="sA", bufs=1)
    sB = ppool.tile([P, (C // 4) * wpp], FP16, tag="sB", bufs=1)
    vA = sA[:].rearrange("p (c w) -> p c w", c=C // 2)
    vB = sB[:].rearrange("p (c w) -> p c w", c=C // 4)
    for out_t, op in ((fmaxh, AL.max), (fminh, AL.min)):
        cur = Fhv
        nch = C
        views = [vA, vB]
        bi = 0
        while nch > 4:
            half = nch // 2
            dst = views[bi % 2][:, :half, :]
            nc.vector.tensor_tensor(dst, cur[:, :half, :], cur[:, half:nch, :], op)
            cur = dst
            nch = half
            bi += 1
        nc.vector.tensor_reduce(
            out_t[:], cur[:, 0:4, :].rearrange("p c w -> p w c"),
            axis=mybir.AxisListType.X, op=op,
        )
    # lm1 = 2**bits - 1 exactly
    lvl_i = ppool.tile([P, wpp], I32, tag="lvl_i")
    nc.vector.tensor_scalar_add(lvl_i[:], bt[:], 127)
    nc.vector.tensor_scalar(lvl_i[:], lvl_i[:], 23, None, AL.logical_shift_left)
    lm1 = ppool.tile([P, wpp], F32, tag="lm1")
    nc.vector.tensor_scalar_add(lm1[:], lvl_i[:].bitcast(F32), -1.0)
    # per-pixel params (f32 [P, wpp]; stats read from fp16)
    rng = ppool.tile([P, wpp], F32, tag="rng")
    nc.vector.scalar_tensor_tensor(
        rng[:], fmaxh[:], 1e-30, fminh[:], AL.add, AL.subtract
    )
    rinv = ppool.tile([P, wpp], F32, tag="rinv")
    nc.vector.reciprocal(rinv[:], rng[:])
    u = ppool.tile([P, wpp], F32, tag="u")
    nc.vector.scalar_tensor_tensor(u[:], lm1[:], 0.0, rinv[:], AL.add, AL.mult)
    c2 = ppool.tile([P, wpp], F32, tag="c2")
    nc.vector.scalar_tensor_tensor(c2[:], u[:], -1.0, fminh[:], AL.mult, AL.mult)
    ilm1 = ppool.tile([P, wpp], F32, tag="ilm1")
    nc.vector.reciprocal(ilm1[:], lm1[:])
    v = ppool.tile([P, wpp], F32, tag="v")
    nc.vector.scalar_tensor_tensor(v[:], rng[:], 0.0, ilm1[:], AL.add, AL.mult)
    vh = ppool.tile([P, wpp], FP16, tag="vh")
    nc.vector.tensor_copy(vh[:], v[:])
    st["u"] = u
    st["c2"] = c2
    st["vh"] = vh
    st["fminh"] = fminh


def _w_pre_round(nc, fpool, st, wpp, fb_bufs):
    """Stage C: per-channel-range *u, +c2 (DVE range || Pool range) + round."""
    F, Fv = st["F"], st["Fv"]
    u, c2 = st["u"], st["c2"]
    k = PREDVE
    # DVE range: both passes on channels [0:k); Pool range: [k:C). The two
    # ranges are independent so the engines run their chains in parallel.
    if k > 0:
        nc.vector.tensor_tensor(
            Fv[:, :k, :], Fv[:, :k, :], _bcast(u, k, wpp), AL.mult
        )
        nc.vector.tensor_tensor(
            Fv[:, :k, :], Fv[:, :k, :], _bcast(c2, k, wpp), AL.add
        )
    if k < C:
        _p1 = nc.gpsimd.tensor_tensor(
            Fv[:, k:, :], Fv[:, k:, :], _bcast(u, C - k, wpp), AL.mult
        )
        _p2 = nc.gpsimd.tensor_tensor(
            Fv[:, k:, :], Fv[:, k:, :], _bcast(c2, C - k, wpp), AL.add
        )
    Q = fpool.tile([P, C * wpp], FP16, tag="Q", bufs=fb_bufs)
    if KROUND == "act":
        nc.scalar.activation(F[:], F[:], ACTF.Copy, bias=M_MAGIC, scale=1.0)
        nc.scalar.activation(Q[:], F[:], ACTF.Copy, bias=-M_MAGIC, scale=1.0)
    else:
        nc.vector.tensor_scalar(Q[:], F[:], M_MAGIC, M_MAGIC, AL.add, AL.subtract)
    st["Q"] = Q


def _w_tails_store(nc, st, out, wpp):
    """Stage D: DVE fp16 tails + SP-ring stores."""
    SB_PX = P * wpp
    b, px0 = st["b"], st["px0"]
    Q = st["Q"]
    Qv = Q[:].rearrange("p (c w) -> p c w", c=C)
    nc.vector.tensor_tensor(Qv, Qv, _bcast(st["vh"], C, wpp), AL.mult)
    nc.vector.tensor_tensor(Qv, Qv, _bcast(st["fminh"], C, wpp), AL.add)
    eng = nc.scalar if KSTORE == "act" else nc.sync
    for cc in range(0, C, CCH):
        dst = out[b, cc : cc + CCH, px0 : px0 + SB_PX]
        dst = dst.rearrange("c (p w) -> p c w", p=P)
        eng.dma_start(out=dst, in_=Qv[:, cc : cc + CCH, :])


def _build_full(nc, tc, feat, bits, out, reps, wpp, f_bufs, fb_bufs, n_sb):
    """Staged software pipeline: loads(k) | stats(k-1) | tails(k-3) | pre(k-2)."""
    with (
        tc.tile_pool(name="fpool", bufs=2) as fpool,
        tc.tile_pool(name="ppool", bufs=2) as ppool,
    ):
        for _rep in range(reps):
            blocks = [(b, s) for b in range(B_LOC) for s in range(n_sb)]
            n = len(blocks)
            pend = {}
            for k in range(n + 3):
                if k < n:
                    b, s = blocks[k]
                    pend[k] = _w_loads(nc, fpool, ppool, feat, bits, b, s,
                                       wpp, f_bufs)
                if k >= 1 and k - 1 < n:
                    _w_stats(nc, fpool, ppool, pend[k - 1], wpp)
                if k >= 3:
                    _w_tails_store(nc, pend[k - 3], out, wpp)
                    del pend[k - 3]
                if k >= 2 and k - 2 < n:
                    _w_pre_round(nc, fpool, pend[k - 2], wpp, fb_bufs)


def build(
    reps: int = 1,
    variant: str = "full",
    timed_loop: int = 0,
    wpp: int = None,
    f_bufs: int = None,
) -> bass.Bass:
    """Build the per-core Bass program.

    reps: python-unrolled repetitions of the whole (idempotent) workload.
    variant: full | dma (bisection aid: loads + cast + stores only).
    timed_loop: if >0, build a timing-only program: internal DRAM tensors
    (no input upload), tiny dummy output, and a hardware For_i loop running
    the workload `timed_loop` times.
    """
    _patch_tile_drain()
    if wpp is None:
        wpp = WPP
    if f_bufs is None:
        f_bufs = F_BUFS
    fb_bufs = FB_BUFS
    n_sb = PX // (P * wpp)
    out_dt = FP16 if variant == "full" else BF16
    nc = bass.Bass()
    if timed_loop:
        feat = nc.dram_tensor("features_i", [B_LOC, C, PX], F32)
        bits = nc.dram_tensor("bits_i", [B_LOC, PX], I32)
        out = nc.dram_tensor("out_i", [B_LOC, C, PX], out_dt)
        dummy = nc.declare_dram_parameter("out", [1, 128], F32, isOutput=True)
    else:
        feat = nc.declare_dram_parameter(
            "features", [B_LOC, C, PX], F32, isOutput=False
        )
        bits = nc.declare_dram_parameter(
            "bit_allocation", [B_LOC, PX], I32, isOutput=False
        )
        out = nc.declare_dram_parameter(
            "out", [B_LOC, C, PX], out_dt, isOutput=True
        )

    with TileContext(nc) as tc:
        loop_cm = tc.For_i(0, timed_loop, 1) if timed_loop else nullcontext()
        with loop_cm:
            if variant == "full":
                _build_full(nc, tc, feat, bits, out, reps, wpp, f_bufs,
                            fb_bufs, n_sb)
            else:
                with (
                    tc.tile_pool(name="fpool", bufs=2) as fpool,
                    tc.tile_pool(name="ppool", bufs=2) as ppool,
                ):
                    for _rep in range(reps):
                        blocks = [
                            (b, s) for b in range(B_LOC) for s in range(n_sb)
                        ]
                        pend = {}
                        for i in range(len(blocks) + BACKLAG):
                            if i < len(blocks):
                                b, s = blocks[i]
                                pend[i] = _front_a(
                                    nc, fpool, ppool, feat, bits, b, s,
                                    variant, wpp, f_bufs, fb_bufs,
                                )
                            j = i - BACKLAG
                            if j >= 0:
                                _back(nc, pend[j], out, variant, wpp)
                                del pend[j]
                            if i < len(blocks):
                                _front_b(nc, pend[i], variant, wpp)
        if timed_loop:
            with tc.tile_pool(name="dpool", bufs=1) as dpool:
                dtile = dpool.tile([1, 128], F32, tag="dummy")
                nc.vector.memset(dtile[:], 0.0)
                nc.sync.dma_start(out=dummy[:], in_=dtile[:])
    _split_sync_waits(nc)
    return nc


_nc_cache: dict = {}


def _get_nc(reps: int = 1, variant: str = "full", timed_loop: int = 0) -> bass.Bass:
    key = (reps, variant, timed_loop)
    if key not in _nc_cache:
        _nc_cache[key] = build(reps, variant, timed_loop)
    return _nc_cache[key]


def _in_maps(features: np.ndarray, bit_allocation: np.ndarray):
    f = np.ascontiguousarray(features, dtype=np.float32).reshape(B_FULL, C, PX)
    ba = np.ascontiguousarray(bit_allocation, dtype=np.int32).reshape(B_FULL, PX)
    maps = []
    for i in range(N_CORES):
        b0 = i * B_LOC
        maps.append(
            {
                "features": f[b0 : b0 + B_LOC],
                "bit_allocation": ba[b0 : b0 + B_LOC],
            }
        )
    return maps


def run(
    features: np.ndarray,
    bit_allocation: np.ndarray,
    reps: int = 1,
    variant: str = "full",
):
    nc = _get_nc(reps, variant)
    maps = _in_maps(features, bit_allocation)
    res = run_bass_kernel_spmd(nc, maps, core_ids=list(range(N_CORES)))
    outs = [
        np.asarray(res.results[i]["out"]).astype(np.float32).reshape(
            B_LOC, C, H, W
        )
        for i in range(N_CORES)
    ]
    return np.concatenate(outs, axis=0)


def run_timed(timed_loop: int, variant: str = "full", reps: int = 1):
    """Run the timing-only program (no input upload); returns nothing useful."""
    nc = _get_nc(reps, variant, timed_loop)
    maps = [{} for _ in range(N_CORES)]
    run_bass_kernel_spmd(nc, maps, core_ids=list(range(N_CORES)))


def kernel(features: np.ndarray, bit_allocation: np.ndarray) -> np.ndarray:
    return run(features, bit_allocation, reps=1)


# revision 32
# speedup vs baseline: 1.4471x; 1.0410x over previous
"""AdaptiveQuantizer Trainium2 kernel (8 NeuronCores, data-parallel over batch).

Math (per pixel (b,h,w), over C=64 channels):
    fmin/fmax = min/max over channels
    rng  = (fmax + 1e-30) - fmin
    lm1  = 2**bits - 1                (exact, via int shift trick)
    u    = lm1 / rng ;  c2 = -u*fmin ;  v = rng / lm1
    w    = u*f + c2                   in [0, lm1]
    r    = round_half_even(w)         via fp32 +M / -M (M = 1.5*2**23)
    out  = v*r + fmin                 (bf16 out; host casts to f32)

Engine assignment (HW-bisection driven; per 1M-elem superblock, 8/core):
  * ACT: f32->fp16 cast of F, then the +M / -M(->fp16) rounding passes.
    (HW-measured: ACT compute is fast ONLY if no HWDGE stores share the
    ACT ring -- mixing them measured ~12x slow. Stores go on the SP ring.)
  * DVE: fp16 min/max tree (tt-max/min 2x_1p packed) + final 4-ch strided
    reduce; per-pixel params; both pre passes on channels [0:PREDVE);
    fp16 tails *v and +fmin (tt broadcast 2x_1p).
  * Pool: both pre passes (*u, +c2) on channels [PREDVE:64) as
    tensor_tensor broadcast (the only Pool ops walrus accepts).
  * Stats/tails/output in fp16: rel L2 err 1.642e-02 (< 2e-2 gate); exact
    f32 stats would cost ~2x stat time (DVE strided/2-stream ops measured
    ~1.6-2x the nominal 1x model).
  * Stage pipeline with one-iteration skew per stage:
    loads(k) | cast+stats(k-1) | pre+round(k-2) | tails+store(k-3).
"""

import os
import sys
from contextlib import nullcontext

for _p in ("/opt/trn_rl_repo", "/root/.axon_site/_ro/trn_rl_repo"):
    if os.path.isdir(_p) and _p not in sys.path:
        sys.path.insert(0, _p)

import numpy as np

import concourse.bass as bass
import concourse.mybir as mybir
from concourse.bass_utils import run_bass_kernel_spmd
from concourse.tile import TileContext
from concourse.vector_clock import ScopedClock

# Problem shapes (hardcoded per spec)
B_FULL, C, H, W = 16, 64, 256, 256
N_CORES = 8
B_LOC = B_FULL // N_CORES  # images per core
PX = H * W                 # pixels per image
P = 128                    # SBUF partitions
WPP = int(os.environ.get("KWPP", "128"))   # pixels per partition per superblock
F_BUFS = int(os.environ.get("KFBUFS", "3"))
FB_BUFS = int(os.environ.get("KFBBUFS", "3"))
FH_BUFS = int(os.environ.get("KFHBUFS", "2"))
CCH = int(os.environ.get("KCCH", "16"))    # channels per DMA chunk
PRE2DVE = int(os.environ.get("KPRE2DVE", "32"))  # +c2 channels on DVE (rest Pool)
PREDVE = int(os.environ.get("KPREDVE", "17"))  # channels with BOTH pre passes on DVE
BACKLAG = int(os.environ.get("KBACKLAG", "2"))
KCAST = os.environ.get("KCAST", "act")     # f32->fp16 cast engine: dve | act
KROUND = os.environ.get("KROUND", "act")   # +M/-M rounding: act | dve
KSTORE = os.environ.get("KSTORE", "sp")    # store HWDGE ring: sp | act
M_MAGIC = 12582912.0       # 1.5*2**23: fp32 "+M" add == round-to-nearest-even
AL = mybir.AluOpType
F32 = mybir.dt.float32
I32 = mybir.dt.int32
BF16 = mybir.dt.bfloat16
FP16 = mybir.dt.float16
ACTF = mybir.ActivationFunctionType

_drain_patched = False


def _patch_tile_drain():
    """This container's walrus accepts only ONE sync wait per TPB_CTRL
    instruction; Tile's final drain carries one wait per ticked proc.
    Split them across multiple drains."""
    global _drain_patched
    if _drain_patched:
        return
    _drain_patched = True

    def _patched(self, tick_clock, wait_clock):
        nc = self.nc
        drain_inst = nc.sync.drain()
        wait_clock.add_sem_waits(
            drain_inst.ins, ScopedClock({None: tick_clock.global_clock})
        )
        si = drain_inst.ins.sync_info
        waits = list(si.on_wait) if (si is not None and si.on_wait) else []
        if len(waits) > 1:
            si.on_wait = waits[:1]
            for wchunk in waits[1:]:
                extra = nc.sync.drain()
                esi = extra.ins.sync_info
                if esi is None:
                    extra.ins.sync_info = mybir.SyncInfo(
                        on_wait=[wchunk], on_update=[]
                    )
                else:
                    esi.on_wait = [wchunk]
        nc.all_engine_barrier()
        assert self.sems is not None
        popped = nc._tile_sem_poison_stack.pop()
        assert popped is self._sem_poison
        nc.clear_and_free_semaphores(list(self.sems.allocated().values()))
        nc.all_engine_barrier()

    TileContext._drain_and_barrier = _patched


def _split_sync_waits(nc: bass.Bass, max_waits: int = 1) -> None:
    """This container's walrus rejects instructions carrying more than one
    sync wait. Hoist excess waits onto injected same-engine NOPs placed
    immediately before the instruction (engine program order makes this
    semantically identical)."""
    k = 0
    for bb in nc.main_func.blocks:
        insts = list(bb.instructions)
        out_list = []
        changed = False
        for inst in insts:
            si = inst.sync_info
            waits = list(si.on_wait) if (si is not None and si.on_wait) else []
            if len(waits) > max_waits:
                keep = waits[-max_waits:]
                hoist = waits[:-max_waits]
                for i in range(0, len(hoist), max_waits):
                    nop = mybir.InstNoOp(name=f"WSPL-{k}", ins=[], outs=[])
                    k += 1
                    nop.engine = inst.engine
                    nop.sync_info = mybir.SyncInfo(
                        on_wait=hoist[i : i + max_waits], on_update=[]
                    )
                    out_list.append(nop)
                si.on_wait = keep
                changed = True
            out_list.append(inst)
        if changed:
            bb.instructions.clear()
            for inst in out_list:
                bb.instructions.append(inst)


def _bcast(t, nch, wpp):
    return t[:].rearrange("p (o w) -> p o w", o=1).to_broadcast((P, nch, wpp))


def _vflags(variant):
    """Stage flags per variant (bisection aids)."""
    full = variant == "full"
    return {
        "stats": full or variant in ("red", "tree"),
        "tree": variant == "tree",
        "pre": "split" if full else (
            "dve" if variant == "pre_dve" else
            "pool" if variant == "pre_pool" else None
        ),
        "round": (
            "act" if full or variant in ("act2", "act2_sp") else
            "dve" if variant == "rounddve" else None
        ),
        "tails": full,
        "sp_store": variant in ("dma_sp", "act2_sp"),
    }


def _emit_stats(nc, ppool, Fv, wpp):
    fmax = ppool.tile([P, wpp], F32, tag="fmax")
    fmin = ppool.tile([P, wpp], F32, tag="fmin")
    tmax = ppool.tile([P, wpp], F32, tag="tmax")
    tmin = ppool.tile([P, wpp], F32, tag="tmin")
    h = C // 2
    nc.vector.tensor_reduce(
        fmax[:], Fv[:, :h, :].rearrange("p c w -> p w c"),
        axis=mybir.AxisListType.X, op=AL.max,
    )
    nc.vector.tensor_reduce(
        fmin[:], Fv[:, :h, :].rearrange("p c w -> p w c"),
        axis=mybir.AxisListType.X, op=AL.min,
    )
    nc.vector.tensor_reduce(
        tmax[:], Fv[:, h:, :].rearrange("p c w -> p w c"),
        axis=mybir.AxisListType.X, op=AL.max,
    )
    nc.vector.tensor_reduce(
        tmin[:], Fv[:, h:, :].rearrange("p c w -> p w c"),
        axis=mybir.AxisListType.X, op=AL.min,
    )
    nc.vector.tensor_tensor(fmax[:], fmax[:], tmax[:], AL.max)
    nc.vector.tensor_tensor(fmin[:], fmin[:], tmin[:], AL.min)
    return fmax, fmin


def _emit_stats_tree(nc, ppool, Fv, wpp):
    """Baseline-style contiguous stt tree (64->32->16->8->4, reduce 4)."""
    fmax = ppool.tile([P, wpp], F32, tag="fmax")
    fmin = ppool.tile([P, wpp], F32, tag="fmin")
    sA = ppool.tile([P, (C // 2) * wpp], F32, tag="sA", bufs=1)
    sB = ppool.tile([P, (C // 4) * wpp], F32, tag="sB", bufs=1)
    vA = sA[:].rearrange("p (c w) -> p c w", c=C // 2)
    vB = sB[:].rearrange("p (c w) -> p c w", c=C // 4)
    for out_t, op in ((fmax, AL.max), (fmin, AL.min)):
        cur = Fv
        nch = C
        views = [vA, vB]
        bi = 0
        while nch > 4:
            half = nch // 2
            dst = views[bi % 2][:, :half, :]
            nc.vector.scalar_tensor_tensor(
                dst, cur[:, :half, :], 0.0, cur[:, half:nch, :], AL.add, op
            )
            cur = dst
            nch = half
            bi += 1
        nc.vector.tensor_reduce(
            out_t[:],
            cur[:, 0:4, :].rearrange("p c w -> p w c"),
            axis=mybir.AxisListType.X,
            op=op,
        )
    return fmax, fmin


def _front_a(nc, fpool, ppool, feat, bits, b, s, variant, wpp, f_bufs,
             fb_bufs):
    """Loads + DVE stats + params + ACT bf16 param copies."""
    fl = _vflags(variant)
    SB_PX = P * wpp
    px0 = s * SB_PX
    F = fpool.tile([P, C * wpp], F32, tag="F", bufs=f_bufs)
    Fv = F[:].rearrange("p (c w) -> p c w", c=C)
    # ---- loads (SP HWDGE ring): 1 MiB chunks, contiguous 512 B runs ----
    for cc in range(0, C, CCH):
        src = feat[b, cc : cc + CCH, px0 : px0 + SB_PX]
        src = src.rearrange("c (p w) -> p c w", p=P)
        nc.sync.dma_start(out=Fv[:, cc : cc + CCH, :], in_=src)
    bt = ppool.tile([P, wpp], I32, tag="bt")
    nc.sync.dma_start(
        out=bt[:],
        in_=bits[b, px0 : px0 + SB_PX].rearrange("(p w) -> p w", p=P),
    )

    Fb = fpool.tile([P, C * wpp], BF16, tag="Fb", bufs=fb_bufs)
    st = {"F": F, "Fv": Fv, "Fb": Fb, "b": b, "px0": px0,
          "vb": None, "fminb": None}
    if variant != "full":
        if fl["pre"] == "dve":
            u = ppool.tile([P, wpp], F32, tag="u")
            c2 = ppool.tile([P, wpp], F32, tag="c2")
            nc.vector.memset(u[:], 1.0)
            nc.vector.memset(c2[:], 0.0)
            nc.vector.tensor_tensor(Fv, Fv, _bcast(u, C, wpp), AL.mult)
            nc.vector.tensor_tensor(Fv, Fv, _bcast(c2, C, wpp), AL.add)
        elif fl["pre"] == "pool":
            u = ppool.tile([P, wpp], F32, tag="u")
            c2 = ppool.tile([P, wpp], F32, tag="c2")
            nc.vector.memset(u[:], 1.0)
            nc.vector.memset(c2[:], 0.0)
            _p1 = nc.gpsimd.tensor_tensor(Fv, Fv, _bcast(u, C, wpp), AL.mult)
            _p2 = nc.gpsimd.tensor_tensor(Fv, Fv, _bcast(c2, C, wpp), AL.add)
        if fl["stats"]:
            if fl["tree"]:
                _emit_stats_tree(nc, ppool, Fv, wpp)
            else:
                _emit_stats(nc, ppool, Fv, wpp)
        if fl["round"] == "act":
            nc.scalar.activation(F[:], F[:], ACTF.Copy, bias=M_MAGIC, scale=1.0)
            nc.scalar.activation(Fb[:], F[:], ACTF.Copy, bias=-M_MAGIC, scale=1.0)
        elif fl["round"] == "dve":
            nc.vector.tensor_scalar(
                Fb[:], F[:], M_MAGIC, M_MAGIC, AL.add, AL.subtract
            )
        else:
            nc.vector.tensor_copy(Fb[:], F[:])
        return st

    # ---- channel min/max: two half-channel strided DVE reduces each, so
    # the first starts as soon as the first two DMA chunks land ----
    fmax, fmin = _emit_stats(nc, ppool, Fv, wpp)

    # ---- lm1 = 2**bits - 1 exactly: (bits+127)<<23 bitcast f32, -1 ----
    lvl_i = ppool.tile([P, wpp], I32, tag="lvl_i")
    nc.vector.tensor_scalar_add(lvl_i[:], bt[:], 127)
    nc.vector.tensor_scalar(lvl_i[:], lvl_i[:], 23, None, AL.logical_shift_left)
    lm1 = ppool.tile([P, wpp], F32, tag="lm1")
    nc.vector.tensor_scalar_add(lm1[:], lvl_i[:].bitcast(F32), -1.0)

    # ---- per-pixel params ([P, wpp] tiles, small DVE ops) ----
    rng = ppool.tile([P, wpp], F32, tag="rng")
    nc.vector.scalar_tensor_tensor(
        rng[:], fmax[:], 1e-30, fmin[:], AL.add, AL.subtract
    )
    rinv = ppool.tile([P, wpp], F32, tag="rinv")
    nc.vector.reciprocal(rinv[:], rng[:])
    u = ppool.tile([P, wpp], F32, tag="u")
    nc.vector.scalar_tensor_tensor(u[:], lm1[:], 0.0, rinv[:], AL.add, AL.mult)
    c2 = ppool.tile([P, wpp], F32, tag="c2")
    nc.vector.scalar_tensor_tensor(c2[:], u[:], -1.0, fmin[:], AL.mult, AL.mult)
    ilm1 = ppool.tile([P, wpp], F32, tag="ilm1")
    nc.vector.reciprocal(ilm1[:], lm1[:])
    v = ppool.tile([P, wpp], F32, tag="v")
    nc.vector.scalar_tensor_tensor(v[:], rng[:], 0.0, ilm1[:], AL.add, AL.mult)
    # bf16 copies of v / fmin for the tails (ACT)
    vb = ppool.tile([P, wpp], BF16, tag="vb")
    nc.scalar.activation(vb[:], v[:], ACTF.Copy, bias=0.0, scale=1.0)
    fminb = ppool.tile([P, wpp], BF16, tag="fminb")
    nc.scalar.activation(fminb[:], fmin[:], ACTF.Copy, bias=0.0, scale=1.0)
    st["u"] = u
    st["c2"] = c2
    st["vb"] = vb
    st["fminb"] = fminb
    return st


def _front_b(nc, st, variant, wpp):
    """Pre-round passes (*u on Pool, +c2 split DVE/Pool) + ACT rounding."""
    if variant != "full":
        return
    F, Fv, Fb = st["F"], st["Fv"], st["Fb"]
    u, c2 = st["u"], st["c2"]
    # *u: Pool tt-mult with broadcast (walrus allows Pool tt add/mult only)
    _p1 = nc.gpsimd.tensor_tensor(Fv, Fv, _bcast(u, C, wpp), AL.mult)
    # +c2: small DVE slice + Pool for the rest, disjoint channel ranges
    k = PRE2DVE
    if k > 0:
        nc.vector.tensor_tensor(
            Fv[:, :k, :], Fv[:, :k, :], _bcast(c2, k, wpp), AL.add
        )
    if k < C:
        _p2 = nc.gpsimd.tensor_tensor(
            Fv[:, k:, :], Fv[:, k:, :], _bcast(c2, C - k, wpp), AL.add
        )
    # ---- rounding on ACT: +M (f32, in place), then -M -> bf16 (exact) ----
    nc.scalar.activation(F[:], F[:], ACTF.Copy, bias=M_MAGIC, scale=1.0)
    nc.scalar.activation(Fb[:], F[:], ACTF.Copy, bias=-M_MAGIC, scale=1.0)


def _back(nc, st, out, variant, wpp):
    """Post-round bf16 tails (DVE 2x tt) + bf16 store on the ACT HWDGE ring."""
    SB_PX = P * wpp
    b, px0 = st["b"], st["px0"]
    Fb = st["Fb"]
    Fbv = Fb[:].rearrange("p (c w) -> p c w", c=C)
    if st["vb"] is not None:
        nc.vector.tensor_tensor(Fbv, Fbv, _bcast(st["vb"], C, wpp), AL.mult)
        nc.vector.tensor_tensor(Fbv, Fbv, _bcast(st["fminb"], C, wpp), AL.add)
    eng = nc.sync if _vflags(variant)["sp_store"] else nc.scalar
    for cc in range(0, C, CCH):
        dst = out[b, cc : cc + CCH, px0 : px0 + SB_PX]
        dst = dst.rearrange("c (p w) -> p c w", p=P)
        eng.dma_start(out=dst, in_=Fbv[:, cc : cc + CCH, :])


def _w_loads(nc, fpool, ppool, feat, bits, b, s, wpp, f_bufs):
    """Stage A: SP-ring loads for superblock (b, s)."""
    SB_PX = P * wpp
    px0 = s * SB_PX
    F = fpool.tile([P, C * wpp], F32, tag="F", bufs=f_bufs)
    Fv = F[:].rearrange("p (c w) -> p c w", c=C)
    for cc in range(0, C, CCH):
        src = feat[b, cc : cc + CCH, px0 : px0 + SB_PX]
        src = src.rearrange("c (p w) -> p c w", p=P)
        nc.sync.dma_start(out=Fv[:, cc : cc + CCH, :], in_=src)
    bt = ppool.tile([P, wpp], I32, tag="bt")
    nc.sync.dma_start(
        out=bt[:],
        in_=bits[b, px0 : px0 + SB_PX].rearrange("(p w) -> p w", p=P),
    )
    return {"F": F, "Fv": Fv, "bt": bt, "b": b, "px0": px0}


def _w_stats(nc, fpool, ppool, st, wpp):
    """Stage B: f32->fp16 cast + DVE fp16 min/max tree + params."""
    F, Fv, bt = st["F"], st["Fv"], st["bt"]
    Fh = fpool.tile([P, C * wpp], FP16, tag="Fh", bufs=FH_BUFS)
    if KCAST == "act":
        nc.scalar.activation(Fh[:], F[:], ACTF.Copy, bias=0.0, scale=1.0)
    else:
        nc.vector.tensor_copy(Fh[:], F[:])
    Fhv = Fh[:].rearrange("p (c w) -> p c w", c=C)
    fmaxh = ppool.tile([P, wpp], FP16, tag="fmaxh")
    fminh = ppool.tile([P, wpp], FP16, tag="fminh")
    sA = ppool.tile([P, (C // 2) * wpp], FP16, tag="sA", bufs=1)
    sB = ppool.tile([P, (C // 4) * wpp], FP16, tag="sB", bufs=1)
    vA = sA[:].rearrange("p (c w) -> p c w", c=C // 2)
    vB = sB[:].rearrange("p (c w) -> p c w", c=C // 4)
    for out_t, op in ((fmaxh, AL.max), (fminh, AL.min)):
        cur = Fhv
        nch = C
        views = [vA, vB]
        bi = 0
        while nch > 4:
            half = nch // 2
            dst = views[bi % 2][:, :half, :]
            nc.vector.tensor_tensor(dst, cur[:, :half, :], cur[:, half:nch, :], op)
            cur = dst
            nch = half
            bi += 1
        nc.vector.tensor_reduce(
            out_t[:], cur[:, 0:4, :].rearrange("p c w -> p w c"),
            axis=mybir.AxisListType.X, op=op,
        )
    # lm1 = 2**bits - 1 exactly
    lvl_i = ppool.tile([P, wpp], I32, tag="lvl_i")
    nc.vector.tensor_scalar_add(lvl_i[:], bt[:], 127)
    nc.vector.tensor_scalar(lvl_i[:], lvl_i[:], 23, None, AL.logical_shift_left)
    lm1 = ppool.tile([P, wpp], F32, tag="lm1")
    nc.vector.tensor_scalar_add(lm1[:], lvl_i[:].bitcast(F32), -1.0)
    # per-pixel params (f32 [P, wpp]; stats read from fp16)
    rng = ppool.tile([P, wpp], F32, tag="rng")
    nc.vector.scalar_tensor_tensor(
        rng[:], fmaxh[:], 1e-30, fminh[:], AL.add, AL.subtract
    )
    rinv = ppool.tile([P, wpp], F32, tag="rinv")
    nc.vector.reciprocal(rinv[:], rng[:])
    u = ppool.tile([P, wpp], F32, tag="u")
    nc.vector.scalar_tensor_tensor(u[:], lm1[:], 0.0, rinv[:], AL.add, AL.mult)
    c2 = ppool.tile([P, wpp], F32, tag="c2")
    nc.vector.scalar_tensor_tensor(c2[:], u[:], -1.0, fminh[:], AL.mult, AL.mult)
    ilm1 = ppool.tile([P, wpp], F32, tag="ilm1")
    nc.vector.reciprocal(ilm1[:], lm1[:])
    v = ppool.tile([P, wpp], F32, tag="v")
    nc.vector.scalar_tensor_tensor(v[:], rng[:], 0.0, ilm1[:], AL.add, AL.mult)
    vh = ppool.tile([P, wpp], FP16, tag="vh")
    nc.vector.tensor_copy(vh[:], v[:])
    st["u"] = u
    st["c2"] = c2
    st["vh"] = vh
    st["fminh"] = fminh


def _w_pre_round(nc, fpool, st, wpp, fb_bufs):
    """Stage C in two half-channel groups: per group, *u and +c2 (DVE slice
    || Pool slice) then +M/-M rounding, so group-0 rounding on ACT overlaps
    group-1 pre passes on Pool."""
    F, Fv = st["F"], st["Fv"]
    u, c2 = st["u"], st["c2"]
    Q = fpool.tile([P, C * wpp], FP16, tag="Q", bufs=fb_bufs)
    Fg = F[:].rearrange("p (g x) -> p g x", g=2)
    Qg = Q[:].rearrange("p (g x) -> p g x", g=2)
    h = C // 2
    kh = PREDVE // 2
    for g in range(2):
        c0 = g * h
        if kh > 0:
            sl = Fv[:, c0 : c0 + kh, :]
            nc.vector.tensor_tensor(sl, sl, _bcast(u, kh, wpp), AL.mult)
            nc.vector.tensor_tensor(sl, sl, _bcast(c2, kh, wpp), AL.add)
        if kh < h:
            sp = Fv[:, c0 + kh : c0 + h, :]
            _p1 = nc.gpsimd.tensor_tensor(
                sp, sp, _bcast(u, h - kh, wpp), AL.mult
            )
            _p2 = nc.gpsimd.tensor_tensor(
                sp, sp, _bcast(c2, h - kh, wpp), AL.add
            )
        if KROUND == "act":
            nc.scalar.activation(
                Fg[:, g, :], Fg[:, g, :], ACTF.Copy, bias=M_MAGIC, scale=1.0
            )
            nc.scalar.activation(
                Qg[:, g, :], Fg[:, g, :], ACTF.Copy, bias=-M_MAGIC, scale=1.0
            )
        else:
            nc.vector.tensor_scalar(
                Qg[:, g, :], Fg[:, g, :], M_MAGIC, M_MAGIC, AL.add, AL.subtract
            )
    st["Q"] = Q


def _w_tails_store(nc, st, out, wpp):
    """Stage D: DVE fp16 tails + SP-ring stores."""
    SB_PX = P * wpp
    b, px0 = st["b"], st["px0"]
    Q = st["Q"]
    Qv = Q[:].rearrange("p (c w) -> p c w", c=C)
    nc.vector.tensor_tensor(Qv, Qv, _bcast(st["vh"], C, wpp), AL.mult)
    nc.vector.tensor_tensor(Qv, Qv, _bcast(st["fminh"], C, wpp), AL.add)
    eng = nc.scalar if KSTORE == "act" else nc.sync
    for cc in range(0, C, CCH):
        dst = out[b, cc : cc + CCH, px0 : px0 + SB_PX]
        dst = dst.rearrange("c (p w) -> p c w", p=P)
        eng.dma_start(out=dst, in_=Qv[:, cc : cc + CCH, :])


def _build_full(nc, tc, feat, bits, out, reps, wpp, f_bufs, fb_bufs, n_sb):
    """Staged software pipeline: loads(k) | stats(k-1) | tails(k-3) | pre(k-2)."""
    with (
        tc.tile_pool(name="fpool", bufs=2) as fpool,
        tc.tile_pool(name="ppool", bufs=2) as ppool,
    ):
        for _rep in range(reps):
            blocks = [(b, s) for b in range(B_LOC) for s in range(n_sb)]
            n = len(blocks)
            pend = {}
            for k in range(n + 3):
                if k < n:
                    b, s = blocks[k]
                    pend[k] = _w_loads(nc, fpool, ppool, feat, bits, b, s,
                                       wpp, f_bufs)
                if k >= 1 and k - 1 < n:
                    _w_stats(nc, fpool, ppool, pend[k - 1], wpp)
                if k >= 3:
                    _w_tails_store(nc, pend[k - 3], out, wpp)
                    del pend[k - 3]
                if k >= 2 and k - 2 < n:
                    _w_pre_round(nc, fpool, pend[k - 2], wpp, fb_bufs)


def build(
    reps: int = 1,
    variant: str = "full",
    timed_loop: int = 0,
    wpp: int = None,
    f_bufs: int = None,
) -> bass.Bass:
    """Build the per-core Bass program.

    reps: python-unrolled repetitions of the whole (idempotent) workload.
    variant: full | dma (bisection aid: loads + cast + stores only).
    timed_loop: if >0, build a timing-only program: internal DRAM tensors
    (no input upload), tiny dummy output, and a hardware For_i loop running
    the workload `timed_loop` times.
    """
    _patch_tile_drain()
    if wpp is None:
        wpp = WPP
    if f_bufs is None:
        f_bufs = F_BUFS
    fb_bufs = FB_BUFS
    n_sb = PX // (P * wpp)
    out_dt = FP16 if variant == "full" else BF16
    nc = bass.Bass()
    if timed_loop:
        feat = nc.dram_tensor("features_i", [B_LOC, C, PX], F32)
        bits = nc.dram_tensor("bits_i", [B_LOC, PX], I32)
        out = nc.dram_tensor("out_i", [B_LOC, C, PX], out_dt)
        dummy = nc.declare_dram_parameter("out", [1, 128], F32, isOutput=True)
    else:
        feat = nc.declare_dram_parameter(
            "features", [B_LOC, C, PX], F32, isOutput=False
        )
        bits = nc.declare_dram_parameter(
            "bit_allocation", [B_LOC, PX], I32, isOutput=False
        )
        out = nc.declare_dram_parameter(
            "out", [B_LOC, C, PX], out_dt, isOutput=True
        )

    with TileContext(nc) as tc:
        loop_cm = tc.For_i(0, timed_loop, 1) if timed_loop else nullcontext()
        with loop_cm:
            if variant == "full":
                _build_full(nc, tc, feat, bits, out, reps, wpp, f_bufs,
                            fb_bufs, n_sb)
            else:
                with (
                    tc.tile_pool(name="fpool", bufs=2) as fpool,
                    tc.tile_pool(name="ppool", bufs=2) as ppool,
                ):
                    for _rep in range(reps):
                        blocks = [
                            (b, s) for b in range(B_LOC) for s in range(n_sb)
                        ]
                        pend = {}
                        for i in range(len(blocks) + BACKLAG):
                            if i < len(blocks):
                                b, s = blocks[i]
                                pend[i] = _front_a(
                                    nc, fpool, ppool, feat, bits, b, s,
                                    variant, wpp, f_bufs, fb_bufs,
                                )
                            j = i - BACKLAG
                            if j >= 0:
                                _back(nc, pend[j], out, variant, wpp)
                                del pend[j]
                            if i < len(blocks):
                                _front_b(nc, pend[i], variant, wpp)
        if timed_loop:
            with tc.tile_pool(name="dpool", bufs=1) as dpool:
                dtile = dpool.tile([1, 128], F32, tag="dummy")
                nc.vector.memset(dtile[:], 0.0)
                nc.sync.dma_start(out=dummy[:], in_=dtile[:])
    _split_sync_waits(nc)
    return nc


_nc_cache: dict = {}


def _get_nc(reps: int = 1, variant: str = "full", timed_loop: int = 0) -> bass.Bass:
    key = (reps, variant, timed_loop)
    if key not in _nc_cache:
        _nc_cache[key] = build(reps, variant, timed_loop)
    return _nc_cache[key]


def _in_maps(features: np.ndarray, bit_allocation: np.ndarray):
    f = np.ascontiguousarray(features, dtype=np.float32).reshape(B_FULL, C, PX)
    ba = np.ascontiguousarray(bit_allocation, dtype=np.int32).reshape(B_FULL, PX)
    maps = []
    for i in range(N_CORES):
        b0 = i * B_LOC
        maps.append(
            {
                "features": f[b0 : b0 + B_LOC],
                "bit_allocation": ba[b0 : b0 + B_LOC],
            }
        )
    return maps


def run(
    features: np.ndarray,
    bit_allocation: np.ndarray,
    reps: int = 1,
    variant: str = "full",
):
    nc = _get_nc(reps, variant)
    maps = _in_maps(features, bit_allocation)
    res = run_bass_kernel_spmd(nc, maps, core_ids=list(range(N_CORES)))
    outs = [
        np.asarray(res.results[i]["out"]).astype(np.float32).reshape(
            B_LOC, C, H, W
        )
        for i in range(N_CORES)
    ]
    return np.concatenate(outs, axis=0)


def run_timed(timed_loop: int, variant: str = "full", reps: int = 1):
    """Run the timing-only program (no input upload); returns nothing useful."""
    nc = _get_nc(reps, variant, timed_loop)
    maps = [{} for _ in range(N_CORES)]
    run_bass_kernel_spmd(nc, maps, core_ids=list(range(N_CORES)))


def kernel(features: np.ndarray, bit_allocation: np.ndarray) -> np.ndarray:
    return run(features, bit_allocation, reps=1)
